# revision 1
# baseline (speedup 1.0000x reference)
"""Multi-head attention (B=2, L=2048, D=1024, H=16) on 8 TRN2 NeuronCores.

Sharding: core c handles batch b=c//4 and heads 4*(c%4) .. 4*(c%4)+3
(tensor-parallel over heads x data-parallel over batch). Each core computes a
partial [L, D] output (its heads' contribution through wo); the host sums the
4 partials per batch and adds bo.

Device-side layout is transpose-free:
  - host feeds x^T [D, L]
  - Q^T/K^T [2*Dh, L] = (wq_pair)^T x^T  (two heads packed -> M=128 matmuls)
  - V [L, 2*Dh] natural, with a ones-column appended per head so the PV matmul
    also accumulates the softmax denominator (row 64 of the PSUM tile)
  - S^T [Lk, Lq] = K^T.T @ Q^T ; exp without max-subtraction (scores bounded);
    causal handled by skipping k-blocks above the diagonal + 4 mask tiles
  - out partial = sum_heads O^T.T @ wo_rows, contracting both heads of a pair
    in one K=128 matmul.
"""

from contextlib import ExitStack

import numpy as np

import concourse.bass as bass
import concourse.mybir as mybir
import concourse.tile as tile
from concourse import bacc
from concourse.bass_utils import run_bass_kernel_spmd

B, L, D, H = 2, 2048, 1024, 16
DH = D // H          # 64
P = 128              # partitions
NPAIR = 2            # head pairs per core (4 heads)
LQB = 512            # Lq block (max f32 moving free dim)
NLQ = L // LQB       # 4
NKB = L // P         # 16 k blocks
KD = D // P          # 8 contraction blocks over D
N_CORES = 8

F32 = mybir.dt.float32
AF = mybir.ActivationFunctionType

# matmul input dtype: float32 (4 cyc/row), float32r or bfloat16 (1 cyc/row
# at N>=256). All matmul-input tiles and the x/weight DRAM params carry this
# dtype; PSUM accumulation is always f32.
MM_DT = mybir.dt.bfloat16


def _mm(ap):
    return ap


def build_module(mm_dt=None, iters=1, s_bufs=2, ot_bufs=1, out_bufs=2, pt_bufs=6, proj_psum_bufs=2):
    global MM_DT
    if mm_dt is not None:
        MM_DT = mm_dt
    nc = bacc.Bacc("TRN2", target_bir_lowering=False, debug=False,
                   num_devices=N_CORES)

    xt = nc.dram_tensor("xt", [D, L], MM_DT, kind="ExternalInput").ap()
    wq = nc.dram_tensor("wq", [D, 2 * P], MM_DT, kind="ExternalInput").ap()
    wk = nc.dram_tensor("wk", [D, 2 * P], MM_DT, kind="ExternalInput").ap()
    wv = nc.dram_tensor("wv", [D, 2 * P], MM_DT, kind="ExternalInput").ap()
    bq = nc.dram_tensor("bq", [2 * P], F32, kind="ExternalInput").ap()
    bk = nc.dram_tensor("bk", [2 * P], F32, kind="ExternalInput").ap()
    bv = nc.dram_tensor("bv", [2 * P], F32, kind="ExternalInput").ap()
    wo = nc.dram_tensor("wo", [2 * P, D], MM_DT, kind="ExternalInput").ap()
    mask = nc.dram_tensor("mask", [4, P, LQB], F32, kind="ExternalInput").ap()
    out = nc.dram_tensor("out", [L, D], F32, kind="ExternalOutput").ap()

    with tile.TileContext(nc) as tc, ExitStack() as ctx:
        if MM_DT != F32:
            ctx.enter_context(
                nc.allow_low_precision(reason="matmul inputs use reduced dtype"))
        consts = ctx.enter_context(tc.tile_pool(name="consts", bufs=1))
        proj_sb = ctx.enter_context(tc.tile_pool(name="proj_sb", bufs=1))
        work = ctx.enter_context(tc.tile_pool(name="work", bufs=pt_bufs))
        outp = ctx.enter_context(tc.tile_pool(name="outp", bufs=3))

        ps_proj = ctx.enter_context(
            tc.tile_pool(name="ps_proj", bufs=proj_psum_bufs, space="PSUM"))
        ps_s = ctx.enter_context(tc.tile_pool(name="ps_s", bufs=s_bufs, space="PSUM"))
        ps_ot = ctx.enter_context(tc.tile_pool(name="ps_ot", bufs=ot_bufs, space="PSUM"))
        ps_bc = ctx.enter_context(tc.tile_pool(name="ps_bc", bufs=1, space="PSUM"))
        ps_out = ctx.enter_context(
            tc.tile_pool(name="ps_out", bufs=out_bufs, space="PSUM"))

        # ---- load constants -------------------------------------------------
        # x^T as 8 tiles [128, 2048]
        xt_sb = []
        for k in range(KD):
            t = consts.tile([P, L], MM_DT, tag=f"xt{k}")
            nc.sync.dma_start(out=t[:], in_=xt[k * P:(k + 1) * P, :])
            xt_sb.append(t)

        # weights: [128, KD, 256] so slice [:, k, 128p:128p+128] is lhsT
        def load_w(w_ap, name):
            t = consts.tile([P, KD, 2 * P], MM_DT, tag=name)
            for k in range(KD):
                nc.sync.dma_start(
                    out=t[:, k, :], in_=w_ap[k * P:(k + 1) * P, :])
            return t

        wq_sb = load_w(wq, "wq")
        wk_sb = load_w(wk, "wk")
        wv_sb = load_w(wv, "wv")

        wo_sb = []
        for p in range(NPAIR):
            t = consts.tile([P, D], MM_DT, tag=f"wo{p}")
            nc.sync.dma_start(out=t[:], in_=wo[p * P:(p + 1) * P, :])
            wo_sb.append(t)

        # per-partition biases [128, NPAIR] for q and k
        bq_sb = consts.tile([P, NPAIR], F32, tag="bq")
        bk_sb = consts.tile([P, NPAIR], F32, tag="bk")
        for p in range(NPAIR):
            nc.sync.dma_start(out=bq_sb[:, p:p + 1],
                              in_=bq[p * P:(p + 1) * P].rearrange("(p o) -> p o", o=1))
            nc.sync.dma_start(out=bk_sb[:, p:p + 1],
                              in_=bk[p * P:(p + 1) * P].rearrange("(p o) -> p o", o=1))

        # bv broadcast across partitions: [128, 256]
        bv_bc = consts.tile([P, 2 * P], F32, tag="bv")
        bv_b = bass.AP(tensor=bv.tensor, offset=bv.offset,
                       ap=[[0, P]] + list(bv.ap))
        nc.gpsimd.dma_start(out=bv_bc[:], in_=bv_b)

        # mask tiles [128, 4, 512]
        mask_sb = consts.tile([P, 4, LQB], F32, tag="mask")
        for m in range(4):
            nc.sync.dma_start(out=mask_sb[:, m, :], in_=mask[m])

        ones_sb = consts.tile([1, DH], MM_DT, tag="ones")
        nc.vector.memset(ones_sb[:], 1.0)

        # ---- per head-pair: projections then attention ------------------\n        # ---- emission groups (Tile derives sync; order = engine order) ------
        # Interleave pair-1 projections into pair-0 attention (fills PE's
        # exp-wait bubbles), and the output projection into pair-1 attention.
        for _it in range(iters):
            qt0 = proj_sb.tile([P, L], MM_DT, tag="qt0")
            qt1 = proj_sb.tile([P, L], MM_DT, tag="qt1")
            kt0 = proj_sb.tile([P, L], MM_DT, tag="kt0")
            kt1 = proj_sb.tile([P, L], MM_DT, tag="kt1")
            vx0 = proj_sb.tile([P, NKB, 2 * DH + 2], MM_DT, tag="v0")
            vx1 = proj_sb.tile([P, NKB, 2 * DH + 2], MM_DT, tag="v1")
            ot0 = proj_sb.tile([P, L], MM_DT, tag="ot0")
            ot1 = proj_sb.tile([P, L], MM_DT, tag="ot1")
            qt_t, kt_t, vx_t, ot_t = [qt0, qt1], [kt0, kt1], [vx0, vx1], [ot0, ot1]

            def proj_groups(p):
                def qk_group(dst, w_sb, b_sb, c):
                    acc = ps_proj.tile([P, LQB], F32, tag="proj")
                    for k in range(KD):
                        nc.tensor.matmul(
                            acc[:],
                            w_sb[:, k, p * P:(p + 1) * P],
                            xt_sb[k][:, c * LQB:(c + 1) * LQB],
                            start=(k == 0), stop=(k == KD - 1))
                    nc.vector.tensor_scalar_add(
                        dst[:, c * LQB:(c + 1) * LQB], acc[:], b_sb[:, p:p + 1])

                def v_group(j):
                    acc = ps_proj.tile([P, 2 * DH], F32, tag="proj")
                    for k in range(KD):
                        nc.tensor.matmul(
                            acc[:],
                            xt_sb[k][:, j * P:(j + 1) * P],
                            wv_sb[:, k, p * P:(p + 1) * P],
                            start=(k == 0), stop=(k == KD - 1))
                    nc.vector.tensor_add(vx_t[p][:, j, 0:DH], acc[:, 0:DH],
                                         bv_bc[:, p * P:p * P + DH])
                    nc.vector.tensor_add(vx_t[p][:, j, DH + 1:2 * DH + 1],
                                         acc[:, DH:2 * DH],
                                         bv_bc[:, p * P + DH:(p + 1) * P])

                yield lambda: nc.vector.memset(vx_t[p][:], 1.0)
                for dst, w_sb, b_sb in ((qt_t[p], wq_sb, bq_sb),
                                        (kt_t[p], wk_sb, bk_sb)):
                    for c in range(NLQ):
                        yield lambda d=dst, w=w_sb, b=b_sb, cc=c: \
                            qk_group(d, w, b, cc)
                for j in range(NKB):
                    yield lambda jj=j: v_group(jj)

            def attn_group(p, h, i):
                qt, kt, vx, ot = qt_t[p], kt_t[p], vx_t[p], ot_t[p]
                hp = h * DH
                vcol = h * (DH + 1)
                otp = ps_ot.tile([DH + 1, LQB], F32, tag="ot")
                njb = 4 * i + 4
                for j in range(njb):
                    s = ps_s.tile([P, LQB], F32, tag="s")
                    nc.tensor.matmul(
                        s[:],
                        kt[hp:hp + DH, j * P:(j + 1) * P],
                        qt[hp:hp + DH, i * LQB:(i + 1) * LQB],
                        start=True, stop=True)
                    pt = work.tile([P, LQB], MM_DT, tag="pt")
                    nc.scalar.activation(pt[:], s[:], AF.Exp,
                                         scale=1.0 / np.sqrt(DH))
                    if j >= 4 * i:
                        nc.vector.tensor_mul(pt[:], pt[:],
                                             mask_sb[:, j - 4 * i, :])
                    nc.tensor.matmul(
                        otp[:],
                        vx[:, j, vcol:vcol + DH + 1],
                        pt[:],
                        start=(j == 0), stop=(j == njb - 1))
                rec = work.tile([1, LQB], MM_DT, tag="rec")
                nc.vector.reciprocal(rec[:], otp[DH:DH + 1, :])
                bc = ps_bc.tile([DH, LQB], F32, tag="bc")
                nc.tensor.matmul(bc[:], ones_sb[:], rec[:],
                                 start=True, stop=True)
                osl = ot[hp:hp + DH, i * LQB:(i + 1) * LQB]
                nc.vector.tensor_copy(osl, otp[0:DH, :])
                nc.vector.tensor_mul(osl, osl, bc[:])

            def outproj_group(l):
                o_sb = outp.tile([P, D], F32, tag="osb")
                for half in range(2):
                    acc = ps_out.tile([P, LQB], F32, tag="out")
                    for p in range(NPAIR):
                        nc.tensor.matmul(
                            acc[:],
                            ot_t[p][:, l * P:(l + 1) * P],
                            wo_sb[p][:, half * LQB:(half + 1) * LQB],
                            start=(p == 0), stop=(p == NPAIR - 1))
                    nc.vector.tensor_copy(o_sb[:, half * LQB:(half + 1) * LQB],
                                          acc[:])
                nc.sync.dma_start(out=out[l * P:(l + 1) * P, :], in_=o_sb[:])

            # sequential emission: Tile's scheduler handles cross-phase
            # overlap better than manual interleaving (measured)
            for p in range(NPAIR):
                for g in proj_groups(p):
                    g()
                for h in range(2):
                    for i in range(NLQ):
                        attn_group(p, h, i)
            for l in range(NKB):
                outproj_group(l)

    nc.compile()
    return nc


_CACHE = {}


def _get_nc(mm_dt=None, iters=1):
    key = (str(mm_dt), iters)
    if key not in _CACHE:
        _CACHE[key] = build_module(mm_dt, iters)
    return _CACHE[key]


def _np_mm_dtype():
    if MM_DT == mybir.dt.bfloat16:
        import ml_dtypes
        return ml_dtypes.bfloat16
    return np.float32


def _make_in_maps(x, causal_mask, wq, bq, wk, bk, wv, bv, wo):
    x = np.asarray(x, np.float32)
    mdt = _np_mm_dtype()
    cm = np.asarray(causal_mask)
    # 4 multiplicative mask tiles [128, 512]: tile m covers k-block j=m within
    # Lq-block i=0 -> tile[p, c] = 0 if mask(q=c, k=128m+p) else 1
    mt = np.empty((4, P, LQB), np.float32)
    for m in range(4):
        mt[m] = (~cm[0, 0, 0:LQB, m * P:(m + 1) * P]).T.astype(np.float32)
    in_maps = []
    for c in range(N_CORES):
        b = c // 4
        g = c % 4
        cols = slice(256 * g, 256 * (g + 1))
        in_maps.append({
            "xt": np.ascontiguousarray(x[b].T).astype(mdt),
            "wq": np.ascontiguousarray(np.asarray(wq, np.float32)[:, cols]).astype(mdt),
            "wk": np.ascontiguousarray(np.asarray(wk, np.float32)[:, cols]).astype(mdt),
            "wv": np.ascontiguousarray(np.asarray(wv, np.float32)[:, cols]).astype(mdt),
            "bq": np.ascontiguousarray(np.asarray(bq, np.float32)[cols]),
            "bk": np.ascontiguousarray(np.asarray(bk, np.float32)[cols]),
            "bv": np.ascontiguousarray(np.asarray(bv, np.float32)[cols]),
            "wo": np.ascontiguousarray(np.asarray(wo, np.float32)[cols, :]).astype(mdt),
            "mask": mt,
        })
    return in_maps


def run(inputs, trace=False, mm_dt=None, iters=1, **kw):
    nc = _get_nc(mm_dt, iters)
    in_maps = _make_in_maps(
        inputs["x"], inputs["causal_mask"], inputs["wq"], inputs["bq"],
        inputs["wk"], inputs["bk"], inputs["wv"], inputs["bv"], inputs["wo"])
    res = run_bass_kernel_spmd(nc, in_maps, list(range(N_CORES)),
                               trace=trace, **kw)
    bo = np.asarray(inputs["bo"], np.float32)
    out = np.zeros((B, L, D), np.float32)
    for c in range(N_CORES):
        out[c // 4] += res.results[c]["out"]
    out += bo[None, None, :]
    return out, res


def kernel(**inputs):
    out, _ = run(inputs)
    return out



# revision 13
# speedup vs baseline: 1.5214x; 1.5214x over previous
"""Multi-head attention (B=2, L=2048, D=1024, H=16) on 8 TRN2 NeuronCores.

Sharding: core c handles batch b=c//4 and heads 4*(c%4) .. 4*(c%4)+3
(tensor-parallel over heads x data-parallel over batch). Each core computes a
partial [L, D] output (its heads' contribution through wo); the host sums the
4 partials per batch and adds bo.

Device-side design:
  - QKV projections run in fp8-e4m3 DoubleRow matmuls (0.5 cyc/row, two
    128-row contractions per instruction). Accuracy is preserved with a
    compensated split prepared on the host: x = xh + xl, w = wh + wl (wl/xl
    are the fp8 quantization residuals), and x@w is computed as
    xh@wh + xl@wh + xh@wl (the dropped xl@wl term is ~1e-3 relative).
    Weights are pre-scaled by 32 so their uniform(-1/32,1/32) range stays in
    fp8 normal range; biases are pre-scaled to match, and the V "ones"
    column carries the same scale so softmax normalization cancels it.
  - Q^T/K^T [2*Dh, L]; S^T [k, q] blocks of [128, 512] with exp on paired
    2-bank PSUM tiles; causal handled by skipping k-blocks above the
    diagonal, shrinking diagonal tiles to their unmasked column range, and
    bf16 multiplicative mask tiles for the intra-block triangles.
  - PV runs transposed-back: O [q, dh] via lhsT = P^T slice (full 128-wide
    stationary, N=65 moving V+ones) which halves PV row count vs the
    [dh, q] orientation and makes the softmax denominator per-partition
    (one reciprocal + tensor_scalar per q-subblock, no PE broadcast).
  - O [q, dh] -> O^T via XBAR DMA transposes (SBUF->SBUF, 14ns/tile).
  - Projections/attention/output are software-pipelined per 512-row q-chunk:
    QK proj of chunk i+1 is emitted between attention head-groups of chunk
    i so the PE stream has work during exp-bound stretches; the output
    projection + DMA of chunk i follows its transposes.
"""

from contextlib import ExitStack

import numpy as np
import ml_dtypes

import concourse.bass as bass
import concourse.mybir as mybir
import concourse.tile as tile
from concourse import bacc
from concourse.bass_utils import run_bass_kernel_spmd

B, L, D, H = 2, 2048, 1024, 16
DH = D // H          # 64
P = 128              # partitions
NPAIR = 2            # head pairs per core (4 heads)
LQB = 512            # q chunk
NLQ = L // LQB       # 4
NKB = L // P         # 16 k blocks
KD = D // P          # 8 contraction blocks over D
N_CORES = 8
WS = 32.0            # host-side weight scale (fp8 range)
SC = (1.0 / np.sqrt(DH)) / (WS * WS)   # exp scale: undo WS^2 in scores

F32 = mybir.dt.float32
BF16 = mybir.dt.bfloat16
FP8 = mybir.dt.float8e4
AF = mybir.ActivationFunctionType
DR = mybir.MatmulPerfMode.DoubleRow
E4M3 = ml_dtypes.float8_e4m3
BF16NP = ml_dtypes.bfloat16


def build_module(iters=1, dbg=False):
    nc = bacc.Bacc("TRN2", target_bir_lowering=False, debug=False,
                   num_devices=N_CORES)
    if dbg:
        dbg_qt = nc.dram_tensor("dbg_qt", [P, L], F32, kind="ExternalOutput").ap()
        dbg_kt = nc.dram_tensor("dbg_kt", [P, L], F32, kind="ExternalOutput").ap()
        dbg_vx = nc.dram_tensor("dbg_vx", [P, NKB, 2, DH + 1], F32,
                                kind="ExternalOutput").ap()
        dbg_osl = nc.dram_tensor("dbg_osl", [P, NKB, P], F32,
                                 kind="ExternalOutput").ap()
        dbg_otT = nc.dram_tensor("dbg_otT", [P, NKB, P], F32,
                                 kind="ExternalOutput").ap()

    xh_d = nc.dram_tensor("xh", [P, KD, L], FP8, kind="ExternalInput").ap()
    xl_d = nc.dram_tensor("xl", [P, KD, L], FP8, kind="ExternalInput").ap()
    # [hl, p, g2, slot, pair, m]
    wq_d = nc.dram_tensor("wq8", [2, P, 4, 2, 2, P], FP8, kind="ExternalInput").ap()
    wk_d = nc.dram_tensor("wk8", [2, P, 4, 2, 2, P], FP8, kind="ExternalInput").ap()
    # [hl, p, g2, slot, ch]
    wv_d = nc.dram_tensor("wv8", [2, P, 4, 2, 2 * P], FP8, kind="ExternalInput").ap()
    wo_d = nc.dram_tensor("wo", [2, P, D], BF16, kind="ExternalInput").ap()
    bq_d = nc.dram_tensor("bq", [P, 2], F32, kind="ExternalInput").ap()
    bk_d = nc.dram_tensor("bk", [P, 2], F32, kind="ExternalInput").ap()
    bv_d = nc.dram_tensor("bv", [2 * P], F32, kind="ExternalInput").ap()
    mask_d = nc.dram_tensor("mask", [P, 4, LQB], BF16, kind="ExternalInput").ap()
    out = nc.dram_tensor("out", [L, D], F32, kind="ExternalOutput").ap()

    with tile.TileContext(nc) as tc, ExitStack() as ctx:
        ctx.enter_context(
            nc.allow_low_precision(reason="fp8/bf16 matmul data path"))
        consts = ctx.enter_context(tc.tile_pool(name="consts", bufs=1))
        pers = ctx.enter_context(tc.tile_pool(name="pers", bufs=1))
        work = ctx.enter_context(tc.tile_pool(name="work", bufs=1))
        ps = ctx.enter_context(tc.tile_pool(name="ps", bufs=1, space="PSUM"))

        # ---- const tiles ---------------------------------------------------
        xh_sb = consts.tile([P, KD, L], FP8, tag="xh")
        xl_sb = consts.tile([P, KD, L], FP8, tag="xl")
        wq_sb = consts.tile([P, 2, 4, 2, 2, P], FP8, tag="wq")
        wk_sb = consts.tile([P, 2, 4, 2, 2, P], FP8, tag="wk")
        wv_sb = consts.tile([P, 2, 4, 2, 2 * P], FP8, tag="wv")
        wo_sb = consts.tile([P, 2, D], BF16, tag="wo")
        bq_sb = consts.tile([P, 2], F32, tag="bq")
        bk_sb = consts.tile([P, 2], F32, tag="bk")
        bv_bc = consts.tile([P, 2 * P], F32, tag="bv")
        mask_sb = consts.tile([P, 4, LQB], BF16, tag="mask")

        # DMA order matters: weights for chunk-0 QK first, then x chunk
        # slabs interleaved with the tensors each chunk unlocks.
        def load_x_chunk(ci, hl):
            sl = slice(ci * LQB, (ci + 1) * LQB)
            src = (xh_d, xl_d)[hl]
            dst = (xh_sb, xl_sb)[hl]
            nc.sync.dma_start(out=dst[:, :, sl], in_=src[:, :, sl])

        nc.sync.dma_start(out=wq_sb[:, 0], in_=wq_d[0])
        load_x_chunk(0, 0)
        nc.sync.dma_start(out=wk_sb[:, 0], in_=wk_d[0])
        nc.sync.dma_start(out=bq_sb[:], in_=bq_d)
        nc.sync.dma_start(out=bk_sb[:], in_=bk_d)
        load_x_chunk(0, 1)
        nc.sync.dma_start(out=wq_sb[:, 1], in_=wq_d[1])
        nc.sync.dma_start(out=wk_sb[:, 1], in_=wk_d[1])
        nc.sync.dma_start(out=wv_sb[:, 0], in_=wv_d[0])
        nc.sync.dma_start(out=wv_sb[:, 1], in_=wv_d[1])
        bv_b = bass.AP(tensor=bv_d.tensor, offset=bv_d.offset,
                       ap=[[0, P]] + list(bv_d.ap))
        nc.gpsimd.dma_start(out=bv_bc[:], in_=bv_b)
        nc.sync.dma_start(out=mask_sb[:], in_=mask_d)
        load_x_chunk(1, 0)
        load_x_chunk(1, 1)
        for pair in range(2):
            nc.sync.dma_start(out=wo_sb[:, pair], in_=wo_d[pair])
        for ci in range(2, NLQ):
            load_x_chunk(ci, 0)
            load_x_chunk(ci, 1)

        # ---- persistent work tiles ----------------------------------------
        qt_t = [pers.tile([P, L], BF16, tag=f"qt{p}", name=f"qt{p}") for p in range(2)]
        kt_t = [pers.tile([P, L], BF16, tag=f"kt{p}", name=f"kt{p}") for p in range(2)]
        vx_t = [pers.tile([P, NKB, 2, DH + 1], BF16, tag=f"vx{p}", name=f"vx{p}")
                for p in range(2)]
        osl_t = [pers.tile([P, NKB, P], BF16, tag=f"osl{p}", name=f"osl{p}") for p in range(2)]
        otT_t = [pers.tile([P, NKB, P], BF16, tag=f"otT{p}", name=f"otT{p}") for p in range(2)]

        for _it in range(iters):
            for pair in range(2):
                nc.gpsimd.memset(vx_t[pair][:], WS)

            # (w-term, x-term) for the compensated product
            TERMS = ((0, xh_sb), (1, xh_sb), (0, xl_sb))

            def qk_proj(pair, which, ci):
                w_sb, b_sb, dst = (
                    (wq_sb, bq_sb, qt_t[pair]) if which == 0
                    else (wk_sb, bk_sb, kt_t[pair]))
                sl = slice(ci * LQB, (ci + 1) * LQB)
                acc = ps.tile([P, LQB], F32, tag="acc", bufs=2)
                n = 0
                for wt, x_sb in TERMS:
                    for g2 in range(4):
                        nc.tensor.matmul(
                            acc[:],
                            w_sb[:, wt, g2, :, pair, :],
                            x_sb[:, 2 * g2:2 * g2 + 2, sl],
                            start=(n == 0), stop=(n == 11), perf_mode=DR)
                        n += 1
                nc.vector.tensor_scalar_add(dst[:, sl], acc[:],
                                            b_sb[:, pair:pair + 1])

            def v_proj(j):
                acc = ps.tile([P, 2 * P], F32, tag="acc", bufs=2)
                jsl = slice(j * P, (j + 1) * P)
                n = 0
                for wt, x_sb in TERMS:
                    for g2 in range(4):
                        nc.tensor.matmul(
                            acc[:],
                            x_sb[:, 2 * g2:2 * g2 + 2, jsl],
                            wv_sb[:, wt, g2, :, :],
                            start=(n == 0), stop=(n == 11), perf_mode=DR)
                        n += 1
                for pair in range(2):
                    for h in range(2):
                        c0 = pair * P + h * DH
                        nc.vector.tensor_add(
                            vx_t[pair][:, j, h, 0:DH],
                            acc[:, c0:c0 + DH], bv_bc[:, c0:c0 + DH])

            def s_group(pair, h, ci, jp):
                """S matmuls + exp (+ masks) for pair-tile jp; returns pt."""
                qt, kt = qt_t[pair], kt_t[pair]
                hp = h * DH
                s = ps.tile([P, 2, LQB], F32, tag="s", bufs=2)
                pt = work.tile([P, 2, LQB], BF16, tag="pt", bufs=6)
                for jj in range(2):
                    j = 2 * jp + jj
                    m = j - 4 * ci
                    off = max(0, m) * P
                    nc.tensor.matmul(
                        s[:, jj, off:LQB],
                        kt[hp:hp + DH, j * P:(j + 1) * P],
                        qt[hp:hp + DH, ci * LQB + off:(ci + 1) * LQB],
                        start=True, stop=True)
                if jp < 2 * ci:
                    nc.scalar.activation(pt[:], s[:], AF.Exp, scale=SC)
                else:
                    for jj in range(2):
                        j = 2 * jp + jj
                        m = j - 4 * ci
                        off = max(0, m) * P
                        nc.scalar.activation(pt[:, jj, off:LQB],
                                             s[:, jj, off:LQB],
                                             AF.Exp, scale=SC)
                        if m >= 0:
                            nc.vector.tensor_mul(
                                pt[:, jj, off:LQB], pt[:, jj, off:LQB],
                                mask_sb[:, m, off:LQB])
                return pt

            def pv_group(pair, h, ci, jp, pt, ot):
                # ot is one PSUM bank: hardware start zeroes the whole bank,
                # so the (head, chunk) group has exactly one start (first
                # matmul) and one stop (last matmul).
                vx = vx_t[pair]
                for jj in range(2):
                    j = 2 * jp + jj
                    m = j - 4 * ci
                    for sb in range(max(0, m), 4):
                        nc.tensor.matmul(
                            ot[:, sb, 0:DH + 1],
                            pt[:, jj, sb * P:(sb + 1) * P],
                            vx[:, j, h, :],
                            start=(j == 0 and sb == 0),
                            stop=(j == 4 * ci + 3 and sb == 3),
                            skip_group_check=True)

            def normalize(pair, h, ci, ot):
                hp = h * DH
                rec = work.tile([P, 4], F32, tag="rec", bufs=2)
                nc.vector.reciprocal(rec[:], ot[:, :, DH])
                for sb in range(4):
                    nc.vector.tensor_scalar_mul(
                        osl_t[pair][:, 4 * ci + sb, hp:hp + DH],
                        ot[:, sb, 0:DH], rec[:, sb:sb + 1])

            def outproj_half(lb, half, osb):
                acc = ps.tile([P, LQB], F32, tag="acc", bufs=2)
                for pair in range(2):
                    nc.tensor.matmul(
                        acc[:],
                        otT_t[pair][:, lb, :],
                        wo_sb[:, pair, half * LQB:(half + 1) * LQB],
                        start=(pair == 0), stop=(pair == 1))
                nc.vector.tensor_copy(osb[:, half * LQB:(half + 1) * LQB],
                                      acc[:])

            def outproj_units(ci):
                units = []
                for lb in range(4 * ci, 4 * ci + 4):
                    osb = work.tile([P, D], F32, tag="osb", bufs=4,
                                    name=f"osb{lb}")
                    for half in range(2):
                        def unit(l=lb, o=osb, hf=half):
                            outproj_half(l, hf, o)
                            nc.sync.dma_start(
                                out=out[l * P:(l + 1) * P,
                                        hf * LQB:(hf + 1) * LQB],
                                in_=o[:, hf * LQB:(hf + 1) * LQB])
                        units.append(unit)
                return units

            # prologue: chunk 0 projections for pair 0 only; pair 1 comes
            # through the filler queue during pair-0 attention
            qk_proj(0, 0, 0)
            qk_proj(0, 1, 0)
            for j in range(4):
                v_proj(j)

            # steady state: per chunk, the two head-streams of each pair are
            # interleaved at pair-tile granularity with PV pipelined one
            # round behind S, and a filler queue (next-chunk projections,
            # prev-chunk output projection) feeds the PE stream's exp-wait
            # windows.
            fillers = [lambda: qk_proj(1, 0, 0), lambda: qk_proj(1, 1, 0)]
            quota = [0.0]

            def drain(slots_left):
                # spread remaining fillers evenly over remaining drain slots
                quota[0] += len(fillers) / max(1.0, slots_left)
                while quota[0] >= 1.0 and fillers:
                    quota[0] -= 1.0
                    fillers.pop(0)()

            for ci in range(NLQ):
                nxt = ci + 1
                if nxt < NLQ:
                    for pair in range(2):
                        fillers.append(lambda p=pair: qk_proj(p, 0, nxt))
                        fillers.append(lambda p=pair: qk_proj(p, 1, nxt))
                    for j in range(4 * nxt, 4 * nxt + 4):
                        fillers.append(lambda jj=j: v_proj(jj))
                nrounds = 2 * ci + 2
                slots = 4 * nrounds
                for pair in range(2):
                    ot_h = [ps.tile([P, 4, P], F32, tag="ot", name=f"ot{h}", bufs=2)
                            for h in range(2)]
                    pt_prev = [None, None]
                    for jp in range(nrounds):
                        for h in range(2):
                            pt = s_group(pair, h, ci, jp)
                            if pt_prev[h] is not None:
                                pv_group(pair, h, ci, jp - 1, pt_prev[h],
                                         ot_h[h])
                            pt_prev[h] = pt
                            drain(slots)
                            slots -= 1
                    for h in range(2):
                        pv_group(pair, h, ci, nrounds - 1, pt_prev[h], ot_h[h])
                        normalize(pair, h, ci, ot_h[h])
                    nc.sync.dma_start(
                        out=otT_t[pair][:, 4 * ci:4 * ci + 4, :],
                        in_=osl_t[pair][:, 4 * ci:4 * ci + 4, :],
                        transpose=True)
                while fillers:
                    fillers.pop(0)()
                fillers.extend(outproj_units(ci))
            while fillers:
                fillers.pop(0)()

        if dbg:
            for name_, src, dst in (("qt", qt_t[0], dbg_qt),
                                    ("kt", kt_t[0], dbg_kt),
                                    ("vx", vx_t[0], dbg_vx),
                                    ("osl", osl_t[0], dbg_osl),
                                    ("otT", otT_t[0], dbg_otT)):
                tmp = work.tile(list(src.shape), F32, tag=f"dbg{name_}",
                                name=f"dbg{name_}")
                nc.vector.tensor_copy(tmp[:], src[:])
                nc.sync.dma_start(out=dst, in_=tmp[:])

    nc.compile()
    return nc


_CACHE = {}


def _get_nc(mm_dt=None, iters=1):
    key = iters
    if key not in _CACHE:
        _CACHE[key] = build_module(iters)
    return _CACHE[key]


def _split_fp8(a):
    hi = a.astype(E4M3)
    lo = (a - hi.astype(np.float32)).astype(E4M3)
    return hi, lo


def _make_in_maps(x, causal_mask, wq, bq, wk, bk, wv, bv, wo):
    x = np.asarray(x, np.float32)
    cm = np.asarray(causal_mask)
    # mask tile m (for k-block j = 4i+m): keep[p, c] = (c >= 128m + p)
    mt = np.empty((P, 4, LQB), np.float32)
    for m in range(4):
        mt[:, m, :] = (~cm[0, 0, 0:LQB, m * P:(m + 1) * P]).T
    mt = mt.astype(BF16NP)

    wq = np.asarray(wq, np.float32)
    wk = np.asarray(wk, np.float32)
    wv = np.asarray(wv, np.float32)
    wo = np.asarray(wo, np.float32)
    bq = np.asarray(bq, np.float32)
    bk = np.asarray(bk, np.float32)
    bv = np.asarray(bv, np.float32)

    in_maps = []
    for c in range(N_CORES):
        b = c // 4
        g = c % 4
        cols = slice(256 * g, 256 * (g + 1))

        xt = np.ascontiguousarray(
            x[b].T.reshape(KD, P, L).transpose(1, 0, 2))
        xhi, xlo = _split_fp8(xt)

        def pack_qk(w):
            # [D, 256] -> [p, g2, slot, pair, m], scaled
            a = (w[:, cols] * WS).reshape(4, 2, P, 2, P).transpose(2, 0, 1, 3, 4)
            hi, lo = _split_fp8(np.ascontiguousarray(a))
            return np.stack([hi, lo])

        def pack_v(w):
            a = (w[:, cols] * WS).reshape(4, 2, P, 2 * P).transpose(2, 0, 1, 3)
            hi, lo = _split_fp8(np.ascontiguousarray(a))
            return np.stack([hi, lo])

        in_maps.append({
            "xh": xhi,
            "xl": xlo,
            "wq8": pack_qk(wq),
            "wk8": pack_qk(wk),
            "wv8": pack_v(wv),
            "wo": np.ascontiguousarray(
                wo[cols, :].reshape(2, P, D)).astype(BF16NP),
            "bq": np.ascontiguousarray((bq[cols] * WS).reshape(2, P).T),
            "bk": np.ascontiguousarray((bk[cols] * WS).reshape(2, P).T),
            "bv": np.ascontiguousarray(bv[cols] * WS),
            "mask": mt,
        })
    return in_maps


def run(inputs, trace=False, mm_dt=None, iters=1, **kw):
    nc = _get_nc(mm_dt, iters)
    in_maps = _make_in_maps(
        inputs["x"], inputs["causal_mask"], inputs["wq"], inputs["bq"],
        inputs["wk"], inputs["bk"], inputs["wv"], inputs["bv"], inputs["wo"])
    res = run_bass_kernel_spmd(nc, in_maps, list(range(N_CORES)),
                               trace=trace, **kw)
    bo = np.asarray(inputs["bo"], np.float32)
    out = np.zeros((B, L, D), np.float32)
    for c in range(N_CORES):
        out[c // 4] += res.results[c]["out"]
    out += bo[None, None, :]
    return out, res


def kernel(**inputs):
    out, _ = run(inputs)
    return out


# revision 21
# speedup vs baseline: 1.5354x; 1.0092x over previous
"""Multi-head attention (B=2, L=2048, D=1024, H=16) on 8 TRN2 NeuronCores.

Sharding: core c handles batch b=c//4 and heads 4*(c%4) .. 4*(c%4)+3
(tensor-parallel over heads x data-parallel over batch). Each core computes a
partial [L, D] output (its heads' contribution through wo); the host sums the
4 partials per batch and adds bo.

Device-side design:
  - QKV projections run in fp8-e4m3 DoubleRow matmuls (0.5 cyc/row, two
    128-row contractions per instruction). Accuracy is preserved with a
    compensated split prepared on the host: x = xh + xl, w = wh + wl (wl/xl
    are the fp8 quantization residuals), and x@w is computed as
    xh@wh + xl@wh + xh@wl (the dropped xl@wl term is ~1e-3 relative).
    Weights are pre-scaled by 32 so their uniform(-1/32,1/32) range stays in
    fp8 normal range; biases are pre-scaled to match, and the V "ones"
    column carries the same scale so softmax normalization cancels it.
  - Q^T/K^T [2*Dh, L]; S^T [k, q] blocks of [128, 512] with exp on paired
    2-bank PSUM tiles; causal handled by skipping k-blocks above the
    diagonal, shrinking diagonal tiles to their unmasked column range, and
    bf16 multiplicative mask tiles for the intra-block triangles.
  - PV runs transposed-back: O [q, dh] via lhsT = P^T slice (full 128-wide
    stationary, N=65 moving V+ones) which halves PV row count vs the
    [dh, q] orientation and makes the softmax denominator per-partition
    (one reciprocal + tensor_scalar per q-subblock, no PE broadcast).
  - O [q, dh] -> O^T via XBAR DMA transposes (SBUF->SBUF, 14ns/tile).
  - Projections/attention/output are software-pipelined per 512-row q-chunk:
    QK proj of chunk i+1 is emitted between attention head-groups of chunk
    i so the PE stream has work during exp-bound stretches; the output
    projection + DMA of chunk i follows its transposes.
"""

from contextlib import ExitStack

import numpy as np
import ml_dtypes

import concourse.bass as bass
import concourse.mybir as mybir
import concourse.tile as tile
from concourse import bacc
from concourse.bass_utils import run_bass_kernel_spmd

B, L, D, H = 2, 2048, 1024, 16
DH = D // H          # 64
P = 128              # partitions
NPAIR = 2            # head pairs per core (4 heads)
LQB = 512            # q chunk
NLQ = L // LQB       # 4
NKB = L // P         # 16 k blocks
KD = D // P          # 8 contraction blocks over D
N_CORES = 8
WS = 32.0            # host-side weight scale (fp8 range)
SC = (1.0 / np.sqrt(DH)) / (WS * WS)   # exp scale: undo WS^2 in scores

F32 = mybir.dt.float32
BF16 = mybir.dt.bfloat16
FP8 = mybir.dt.float8e4
AF = mybir.ActivationFunctionType
DR = mybir.MatmulPerfMode.DoubleRow
E4M3 = ml_dtypes.float8_e4m3
BF16NP = ml_dtypes.bfloat16


def build_module(iters=1, dbg=False):
    nc = bacc.Bacc("TRN2", target_bir_lowering=False, debug=False,
                   num_devices=N_CORES)
    if dbg:
        dbg_qt = nc.dram_tensor("dbg_qt", [P, L], F32, kind="ExternalOutput").ap()
        dbg_kt = nc.dram_tensor("dbg_kt", [P, L], F32, kind="ExternalOutput").ap()
        dbg_vx = nc.dram_tensor("dbg_vx", [P, NKB, 2, DH + 1], F32,
                                kind="ExternalOutput").ap()
        dbg_osl = nc.dram_tensor("dbg_osl", [P, NKB, P], F32,
                                 kind="ExternalOutput").ap()
        dbg_otT = nc.dram_tensor("dbg_otT", [P, NKB, P], F32,
                                 kind="ExternalOutput").ap()

    xh_d = nc.dram_tensor("xh", [P, KD, L], FP8, kind="ExternalInput").ap()
    xl_d = nc.dram_tensor("xl", [P, KD, L], FP8, kind="ExternalInput").ap()
    # [hl, p, g2, slot, pair, m]
    wq_d = nc.dram_tensor("wq8", [2, P, 4, 2, 2, P], FP8, kind="ExternalInput").ap()
    wk_d = nc.dram_tensor("wk8", [2, P, 4, 2, 2, P], FP8, kind="ExternalInput").ap()
    # [hl, p, g2, slot, ch]
    wv_d = nc.dram_tensor("wv8", [2, P, 4, 2, 2 * P], FP8, kind="ExternalInput").ap()
    wo_d = nc.dram_tensor("wo", [2, P, D], BF16, kind="ExternalInput").ap()
    bq_d = nc.dram_tensor("bq", [P, 2], F32, kind="ExternalInput").ap()
    bk_d = nc.dram_tensor("bk", [P, 2], F32, kind="ExternalInput").ap()
    bv_d = nc.dram_tensor("bv", [2 * P], F32, kind="ExternalInput").ap()
    mask_d = nc.dram_tensor("mask", [P, 4, LQB], BF16, kind="ExternalInput").ap()
    out = nc.dram_tensor("out", [L, D], F32, kind="ExternalOutput").ap()

    with tile.TileContext(nc) as tc, ExitStack() as ctx:
        ctx.enter_context(
            nc.allow_low_precision(reason="fp8/bf16 matmul data path"))
        consts = ctx.enter_context(tc.tile_pool(name="consts", bufs=1))
        pers = ctx.enter_context(tc.tile_pool(name="pers", bufs=1))
        work = ctx.enter_context(tc.tile_pool(name="work", bufs=1))
        ps = ctx.enter_context(tc.tile_pool(name="ps", bufs=1, space="PSUM"))

        # ---- const tiles ---------------------------------------------------
        xh_sb = consts.tile([P, KD, L], FP8, tag="xh")
        xl_sb = consts.tile([P, KD, L], FP8, tag="xl")
        wq_sb = consts.tile([P, 2, 4, 2, 2, P], FP8, tag="wq")
        wk_sb = consts.tile([P, 2, 4, 2, 2, P], FP8, tag="wk")
        wv_sb = consts.tile([P, 2, 4, 2, 2 * P], FP8, tag="wv")
        wo_sb = consts.tile([P, 2, D], BF16, tag="wo")
        bq_sb = consts.tile([P, 2], F32, tag="bq")
        bk_sb = consts.tile([P, 2], F32, tag="bk")
        bv_bc = consts.tile([P, 2 * P], F32, tag="bv")
        mask_sb = consts.tile([P, 4, LQB], BF16, tag="mask")

        # DMA order matters: weights for chunk-0 QK first, then x chunk
        # slabs interleaved with the tensors each chunk unlocks.
        def load_x_chunk(ci, hl):
            sl = slice(ci * LQB, (ci + 1) * LQB)
            src = (xh_d, xl_d)[hl]
            dst = (xh_sb, xl_sb)[hl]
            nc.sync.dma_start(out=dst[:, :, sl], in_=src[:, :, sl])

        nc.sync.dma_start(out=wq_sb[:, 0], in_=wq_d[0])
        load_x_chunk(0, 0)
        nc.sync.dma_start(out=wk_sb[:, 0], in_=wk_d[0])
        nc.sync.dma_start(out=bq_sb[:], in_=bq_d)
        nc.sync.dma_start(out=bk_sb[:], in_=bk_d)
        load_x_chunk(0, 1)
        nc.sync.dma_start(out=wq_sb[:, 1], in_=wq_d[1])
        nc.sync.dma_start(out=wk_sb[:, 1], in_=wk_d[1])
        nc.sync.dma_start(out=wv_sb[:, 0], in_=wv_d[0])
        nc.sync.dma_start(out=wv_sb[:, 1], in_=wv_d[1])
        bv_b = bass.AP(tensor=bv_d.tensor, offset=bv_d.offset,
                       ap=[[0, P]] + list(bv_d.ap))
        nc.gpsimd.dma_start(out=bv_bc[:], in_=bv_b)
        nc.sync.dma_start(out=mask_sb[:], in_=mask_d)
        load_x_chunk(1, 0)
        load_x_chunk(1, 1)
        for pair in range(2):
            nc.sync.dma_start(out=wo_sb[:, pair], in_=wo_d[pair])
        for ci in range(2, NLQ):
            load_x_chunk(ci, 0)
            load_x_chunk(ci, 1)

        # ---- persistent work tiles ----------------------------------------
        qt_t = [pers.tile([P, L], BF16, tag=f"qt{p}", name=f"qt{p}") for p in range(2)]
        kt_t = [pers.tile([P, L], BF16, tag=f"kt{p}", name=f"kt{p}") for p in range(2)]
        vx_t = [pers.tile([P, NKB, 2, DH + 1], BF16, tag=f"vx{p}", name=f"vx{p}")
                for p in range(2)]
        osl_t = [pers.tile([P, NKB, P], BF16, tag=f"osl{p}", name=f"osl{p}") for p in range(2)]
        otT_t = [pers.tile([P, NKB, P], BF16, tag=f"otT{p}", name=f"otT{p}") for p in range(2)]

        for _it in range(iters):
            for pair in range(2):
                nc.gpsimd.memset(vx_t[pair][:], WS)

            # (w-term, x-term) for the compensated product
            TERMS = ((0, xh_sb), (1, xh_sb), (0, xl_sb))

            def qk_proj(pair, which, ci):
                w_sb, b_sb, dst = (
                    (wq_sb, bq_sb, qt_t[pair]) if which == 0
                    else (wk_sb, bk_sb, kt_t[pair]))
                sl = slice(ci * LQB, (ci + 1) * LQB)
                acc = ps.tile([P, LQB], F32, tag="acc", bufs=2)
                n = 0
                for wt, x_sb in TERMS:
                    for g2 in range(4):
                        nc.tensor.matmul(
                            acc[:],
                            w_sb[:, wt, g2, :, pair, :],
                            x_sb[:, 2 * g2:2 * g2 + 2, sl],
                            start=(n == 0), stop=(n == 11), perf_mode=DR)
                        n += 1
                nc.vector.tensor_scalar_add(dst[:, sl], acc[:],
                                            b_sb[:, pair:pair + 1])

            def v_proj(j):
                acc = ps.tile([P, 2 * P], F32, tag="acc", bufs=2)
                jsl = slice(j * P, (j + 1) * P)
                n = 0
                for wt, x_sb in TERMS:
                    for g2 in range(4):
                        nc.tensor.matmul(
                            acc[:],
                            x_sb[:, 2 * g2:2 * g2 + 2, jsl],
                            wv_sb[:, wt, g2, :, :],
                            start=(n == 0), stop=(n == 11), perf_mode=DR)
                        n += 1
                for pair in range(2):
                    for h in range(2):
                        c0 = pair * P + h * DH
                        nc.vector.tensor_add(
                            vx_t[pair][:, j, h, 0:DH],
                            acc[:, c0:c0 + DH], bv_bc[:, c0:c0 + DH])

            def s_group(pair, h, ci, jp):
                """S matmuls + exp (+ masks) for pair-tile jp; returns pt."""
                qt, kt = qt_t[pair], kt_t[pair]
                hp = h * DH
                s = ps.tile([P, 2, LQB], F32, tag="s", bufs=2)
                pt = work.tile([P, 2, LQB], BF16, tag="pt", bufs=6)
                for jj in range(2):
                    j = 2 * jp + jj
                    m = j - 4 * ci
                    off = max(0, m) * P
                    nc.tensor.matmul(
                        s[:, jj, off:LQB],
                        kt[hp:hp + DH, j * P:(j + 1) * P],
                        qt[hp:hp + DH, ci * LQB + off:(ci + 1) * LQB],
                        start=True, stop=True)
                if jp < 2 * ci:
                    nc.scalar.activation(pt[:], s[:], AF.Exp, scale=SC)
                else:
                    for jj in range(2):
                        j = 2 * jp + jj
                        m = j - 4 * ci
                        off = max(0, m) * P
                        nc.scalar.activation(pt[:, jj, off:LQB],
                                             s[:, jj, off:LQB],
                                             AF.Exp, scale=SC)
                        if m >= 0:
                            nc.vector.tensor_mul(
                                pt[:, jj, off:LQB], pt[:, jj, off:LQB],
                                mask_sb[:, m, off:LQB])
                return pt

            def pv_group(pair, h, ci, jp, pt, ot):
                # ot is one PSUM bank: hardware start zeroes the whole bank,
                # so the (head, chunk) group has exactly one start (first
                # matmul) and one stop (last matmul).
                vx = vx_t[pair]
                for jj in range(2):
                    j = 2 * jp + jj
                    m = j - 4 * ci
                    for sb in range(max(0, m), 4):
                        nc.tensor.matmul(
                            ot[:, sb, 0:DH + 1],
                            pt[:, jj, sb * P:(sb + 1) * P],
                            vx[:, j, h, :],
                            start=(j == 0 and sb == 0),
                            stop=(j == 4 * ci + 3 and sb == 3),
                            skip_group_check=True)

            def normalize(pair, h, ci, ot):
                hp = h * DH
                rec = work.tile([P, 4], F32, tag="rec", bufs=2)
                nc.vector.reciprocal(rec[:], ot[:, :, DH])
                for sb in range(4):
                    nc.vector.tensor_scalar_mul(
                        osl_t[pair][:, 4 * ci + sb, hp:hp + DH],
                        ot[:, sb, 0:DH], rec[:, sb:sb + 1])

            def outproj_half(lb, half, osb):
                acc = ps.tile([P, LQB], F32, tag="acc", bufs=2)
                for pair in range(2):
                    nc.tensor.matmul(
                        acc[:],
                        otT_t[pair][:, lb, :],
                        wo_sb[:, pair, half * LQB:(half + 1) * LQB],
                        start=(pair == 0), stop=(pair == 1))
                nc.vector.tensor_copy(osb[:, half * LQB:(half + 1) * LQB],
                                      acc[:])

            def outproj_units(ci):
                units = []
                for lb in range(4 * ci, 4 * ci + 4):
                    osb = work.tile([P, D], F32, tag="osb", bufs=4,
                                    name=f"osb{lb}")
                    for half in range(2):
                        def unit(l=lb, o=osb, hf=half):
                            outproj_half(l, hf, o)
                            nc.sync.dma_start(
                                out=out[l * P:(l + 1) * P,
                                        hf * LQB:(hf + 1) * LQB],
                                in_=o[:, hf * LQB:(hf + 1) * LQB])
                        units.append(unit)
                return units

            # prologue: chunk 0 projections for pair 0 only; pair 1 comes
            # through the filler queue during pair-0 attention. Q and K are
            # staged hi-terms-first so K's hi matmuls overlap the xl DMA.
            pro_accs = []
            for which in range(2):
                w_sb = (wq_sb, wk_sb)[which]
                acc = ps.tile([P, LQB], F32, tag="acc", bufs=2,
                              name=f"proacc{which}")
                for g2 in range(4):
                    nc.tensor.matmul(
                        acc[:], w_sb[:, 0, g2, :, 0, :],
                        xh_sb[:, 2 * g2:2 * g2 + 2, 0:LQB],
                        start=(g2 == 0), stop=False, perf_mode=DR)
                pro_accs.append(acc)
            for which in range(2):
                w_sb, b_sb, dst = ((wq_sb, bq_sb, qt_t[0]),
                                   (wk_sb, bk_sb, kt_t[0]))[which]
                acc = pro_accs[which]
                n = 0
                for wt, x_sb in ((1, xh_sb), (0, xl_sb)):
                    for g2 in range(4):
                        nc.tensor.matmul(
                            acc[:], w_sb[:, wt, g2, :, 0, :],
                            x_sb[:, 2 * g2:2 * g2 + 2, 0:LQB],
                            start=False, stop=(n == 7), perf_mode=DR)
                        n += 1
                nc.vector.tensor_scalar_add(dst[:, 0:LQB], acc[:],
                                            b_sb[:, 0:1])
            for j in range(4):
                v_proj(j)

            # steady state: per chunk, the two head-streams of each pair are
            # interleaved at pair-tile granularity with PV pipelined one
            # round behind S, and a filler queue (next-chunk projections,
            # prev-chunk output projection) feeds the PE stream's exp-wait
            # windows.
            fillers = [lambda: qk_proj(1, 0, 0), lambda: qk_proj(1, 1, 0)]
            quota = [0.0]

            def drain(slots_left):
                # spread remaining fillers evenly over remaining drain slots
                quota[0] += len(fillers) / max(1.0, slots_left)
                while quota[0] >= 1.0 and fillers:
                    quota[0] -= 1.0
                    fillers.pop(0)()

            for ci in range(NLQ):
                nxt = ci + 1
                if nxt < NLQ:
                    for pair in range(2):
                        fillers.append(lambda p=pair: qk_proj(p, 0, nxt))
                        fillers.append(lambda p=pair: qk_proj(p, 1, nxt))
                    for j in range(4 * nxt, 4 * nxt + 4):
                        fillers.append(lambda jj=j: v_proj(jj))
                nrounds = 2 * ci + 2
                slots = 4 * nrounds
                for pair in range(2):
                    ot_h = [ps.tile([P, 4, P], F32, tag="ot", name=f"ot{h}", bufs=2)
                            for h in range(2)]
                    pt_prev = [None, None]
                    for jp in range(nrounds):
                        for h in range(2):
                            pt = s_group(pair, h, ci, jp)
                            if pt_prev[h] is not None:
                                pv_group(pair, h, ci, jp - 1, pt_prev[h],
                                         ot_h[h])
                            pt_prev[h] = pt
                            drain(slots)
                            slots -= 1
                    for h in range(2):
                        pv_group(pair, h, ci, nrounds - 1, pt_prev[h], ot_h[h])
                        normalize(pair, h, ci, ot_h[h])
                    for qb in range(4 * ci, 4 * ci + 2):
                        nc.sync.dma_start(out=otT_t[pair][:, qb, :],
                                          in_=osl_t[pair][:, qb, :],
                                          transpose=True)
                    nc.sync.dma_start(
                        out=otT_t[pair][:, 4 * ci + 2:4 * ci + 4, :],
                        in_=osl_t[pair][:, 4 * ci + 2:4 * ci + 4, :],
                        transpose=True)
                while fillers:
                    fillers.pop(0)()
                fillers.extend(outproj_units(ci))
            while fillers:
                fillers.pop(0)()

        if dbg:
            for name_, src, dst in (("qt", qt_t[0], dbg_qt),
                                    ("kt", kt_t[0], dbg_kt),
                                    ("vx", vx_t[0], dbg_vx),
                                    ("osl", osl_t[0], dbg_osl),
                                    ("otT", otT_t[0], dbg_otT)):
                tmp = work.tile(list(src.shape), F32, tag=f"dbg{name_}",
                                name=f"dbg{name_}")
                nc.vector.tensor_copy(tmp[:], src[:])
                nc.sync.dma_start(out=dst, in_=tmp[:])

    nc.compile()
    return nc


_CACHE = {}


def _get_nc(mm_dt=None, iters=1):
    key = iters
    if key not in _CACHE:
        _CACHE[key] = build_module(iters)
    return _CACHE[key]


def _split_fp8(a):
    hi = a.astype(E4M3)
    lo = (a - hi.astype(np.float32)).astype(E4M3)
    return hi, lo


def _make_in_maps(x, causal_mask, wq, bq, wk, bk, wv, bv, wo):
    x = np.asarray(x, np.float32)
    cm = np.asarray(causal_mask)
    # mask tile m (for k-block j = 4i+m): keep[p, c] = (c >= 128m + p)
    mt = np.empty((P, 4, LQB), np.float32)
    for m in range(4):
        mt[:, m, :] = (~cm[0, 0, 0:LQB, m * P:(m + 1) * P]).T
    mt = mt.astype(BF16NP)

    wq = np.asarray(wq, np.float32)
    wk = np.asarray(wk, np.float32)
    wv = np.asarray(wv, np.float32)
    wo = np.asarray(wo, np.float32)
    bq = np.asarray(bq, np.float32)
    bk = np.asarray(bk, np.float32)
    bv = np.asarray(bv, np.float32)

    in_maps = []
    for c in range(N_CORES):
        b = c // 4
        g = c % 4
        cols = slice(256 * g, 256 * (g + 1))

        xt = np.ascontiguousarray(
            x[b].T.reshape(KD, P, L).transpose(1, 0, 2))
        xhi, xlo = _split_fp8(xt)

        def pack_qk(w):
            # [D, 256] -> [p, g2, slot, pair, m], scaled
            a = (w[:, cols] * WS).reshape(4, 2, P, 2, P).transpose(2, 0, 1, 3, 4)
            hi, lo = _split_fp8(np.ascontiguousarray(a))
            return np.stack([hi, lo])

        def pack_v(w):
            a = (w[:, cols] * WS).reshape(4, 2, P, 2 * P).transpose(2, 0, 1, 3)
            hi, lo = _split_fp8(np.ascontiguousarray(a))
            return np.stack([hi, lo])

        in_maps.append({
            "xh": xhi,
            "xl": xlo,
            "wq8": pack_qk(wq),
            "wk8": pack_qk(wk),
            "wv8": pack_v(wv),
            "wo": np.ascontiguousarray(
                wo[cols, :].reshape(2, P, D)).astype(BF16NP),
            "bq": np.ascontiguousarray((bq[cols] * WS).reshape(2, P).T),
            "bk": np.ascontiguousarray((bk[cols] * WS).reshape(2, P).T),
            "bv": np.ascontiguousarray(bv[cols] * WS),
            "mask": mt,
        })
    return in_maps


def run(inputs, trace=False, mm_dt=None, iters=1, **kw):
    nc = _get_nc(mm_dt, iters)
    in_maps = _make_in_maps(
        inputs["x"], inputs["causal_mask"], inputs["wq"], inputs["bq"],
        inputs["wk"], inputs["bk"], inputs["wv"], inputs["bv"], inputs["wo"])
    res = run_bass_kernel_spmd(nc, in_maps, list(range(N_CORES)),
                               trace=trace, **kw)
    bo = np.asarray(inputs["bo"], np.float32)
    out = np.zeros((B, L, D), np.float32)
    for c in range(N_CORES):
        out[c // 4] += res.results[c]["out"]
    out += bo[None, None, :]
    return out, res


def kernel(**inputs):
    out, _ = run(inputs)
    return out


# revision 29
# speedup vs baseline: 1.6127x; 1.0504x over previous
"""Multi-head attention (B=2, L=2048, D=1024, H=16) on 8 TRN2 NeuronCores.

Sharding: core c handles batch b=c//4 and heads 4*(c%4) .. 4*(c%4)+3
(tensor-parallel over heads x data-parallel over batch). Each core computes a
partial [L, D] output (its heads' contribution through wo); the host sums the
4 partials per batch and adds bo.

Device-side design:
  - QKV projections run in fp8-e4m3 DoubleRow matmuls (0.5 cyc/row, two
    128-row contractions per instruction). Accuracy is preserved with a
    compensated split prepared on the host: x = xh + xl, w = wh + wl (wl/xl
    are the fp8 quantization residuals), and x@w is computed as
    xh@wh + xl@wh + xh@wl (the dropped xl@wl term is ~1e-3 relative).
    Weights are pre-scaled by 32 so their uniform(-1/32,1/32) range stays in
    fp8 normal range; biases are pre-scaled to match, and the V "ones"
    column carries the same scale so softmax normalization cancels it.
  - Q^T/K^T [2*Dh, L]; S^T [k, q] blocks of [128, 512] with exp on paired
    2-bank PSUM tiles; causal handled by skipping k-blocks above the
    diagonal, shrinking diagonal tiles to their unmasked column range, and
    bf16 multiplicative mask tiles for the intra-block triangles.
  - PV runs transposed-back: O [q, dh] via lhsT = P^T slice (full 128-wide
    stationary, N=65 moving V+ones) which halves PV row count vs the
    [dh, q] orientation and makes the softmax denominator per-partition
    (one reciprocal + tensor_scalar per q-subblock, no PE broadcast).
  - O [q, dh] -> O^T via XBAR DMA transposes (SBUF->SBUF, 14ns/tile).
  - Projections/attention/output are software-pipelined per 512-row q-chunk:
    QK proj of chunk i+1 is emitted between attention head-groups of chunk
    i so the PE stream has work during exp-bound stretches; the output
    projection + DMA of chunk i follows its transposes.
"""

from contextlib import ExitStack

import numpy as np
import ml_dtypes

import concourse.bass as bass
import concourse.mybir as mybir
import concourse.tile as tile
from concourse import bacc
from concourse.bass_utils import run_bass_kernel_spmd

B, L, D, H = 2, 2048, 1024, 16
DH = D // H          # 64
P = 128              # partitions
NPAIR = 2            # head pairs per core (4 heads)
LQB = 512            # q chunk
NLQ = L // LQB       # 4
NKB = L // P         # 16 k blocks
KD = D // P          # 8 contraction blocks over D
N_CORES = 8
WS = 32.0            # host-side weight scale (fp8 range)
SC = (1.0 / np.sqrt(DH)) / (WS * WS)   # exp scale: undo WS^2 in scores

F32 = mybir.dt.float32
BF16 = mybir.dt.bfloat16
FP8 = mybir.dt.float8e4
AF = mybir.ActivationFunctionType
DR = mybir.MatmulPerfMode.DoubleRow
E4M3 = ml_dtypes.float8_e4m3
BF16NP = ml_dtypes.bfloat16


def build_module(iters=1, dbg=False):
    nc = bacc.Bacc("TRN2", target_bir_lowering=False, debug=False,
                   num_devices=N_CORES)
    if dbg:
        dbg_qt = nc.dram_tensor("dbg_qt", [P, L], F32, kind="ExternalOutput").ap()
        dbg_kt = nc.dram_tensor("dbg_kt", [P, L], F32, kind="ExternalOutput").ap()
        dbg_vx = nc.dram_tensor("dbg_vx", [P, NKB, 2, DH + 1], F32,
                                kind="ExternalOutput").ap()
        dbg_osl = nc.dram_tensor("dbg_osl", [P, NKB, P], F32,
                                 kind="ExternalOutput").ap()
        dbg_otT = nc.dram_tensor("dbg_otT", [P, NKB, P], F32,
                                 kind="ExternalOutput").ap()

    xh_d = nc.dram_tensor("xh", [P, KD, L], FP8, kind="ExternalInput").ap()
    xl_d = nc.dram_tensor("xl", [P, KD, L], FP8, kind="ExternalInput").ap()
    # [hl, p, g2, slot, pair, m]
    wq_d = nc.dram_tensor("wq8", [2, P, 4, 2, 2, P], FP8, kind="ExternalInput").ap()
    wk_d = nc.dram_tensor("wk8", [2, P, 4, 2, 2, P], FP8, kind="ExternalInput").ap()
    # [hl, p, g2, slot, ch]
    wv_d = nc.dram_tensor("wv8", [2, P, 4, 2, 2 * P], FP8, kind="ExternalInput").ap()
    wo_d = nc.dram_tensor("wo", [2, P, D], BF16, kind="ExternalInput").ap()
    bq_d = nc.dram_tensor("bq", [P, 2], F32, kind="ExternalInput").ap()
    bk_d = nc.dram_tensor("bk", [P, 2], F32, kind="ExternalInput").ap()
    bv_d = nc.dram_tensor("bv", [2 * P], F32, kind="ExternalInput").ap()
    mask_d = nc.dram_tensor("mask", [P, 4, LQB], BF16, kind="ExternalInput").ap()
    out = nc.dram_tensor("out", [L, D], BF16, kind="ExternalOutput").ap()

    with tile.TileContext(nc) as tc, ExitStack() as ctx:
        ctx.enter_context(
            nc.allow_low_precision(reason="fp8/bf16 matmul data path"))
        consts = ctx.enter_context(tc.tile_pool(name="consts", bufs=1))
        pers = ctx.enter_context(tc.tile_pool(name="pers", bufs=1))
        work = ctx.enter_context(tc.tile_pool(name="work", bufs=1))
        ps = ctx.enter_context(tc.tile_pool(name="ps", bufs=1, space="PSUM"))

        # ---- const tiles ---------------------------------------------------
        xh_sb = consts.tile([P, KD, L], FP8, tag="xh")
        xl_sb = consts.tile([P, KD, L], FP8, tag="xl")
        wq_sb = consts.tile([P, 2, 4, 2, 2, P], FP8, tag="wq")
        wk_sb = consts.tile([P, 2, 4, 2, 2, P], FP8, tag="wk")
        wv_sb = consts.tile([P, 2, 4, 2, 2 * P], FP8, tag="wv")
        wo_sb = consts.tile([P, 2, D], BF16, tag="wo")
        bq_sb = consts.tile([P, 2], F32, tag="bq")
        bk_sb = consts.tile([P, 2], F32, tag="bk")
        bv_bc = consts.tile([P, 2 * P], F32, tag="bv")
        mask_sb = consts.tile([P, 4, LQB], BF16, tag="mask")

        # DMA order matters: weights for chunk-0 QK first, then x chunk
        # slabs interleaved with the tensors each chunk unlocks.
        def load_x_chunk(ci, hl):
            sl = slice(ci * LQB, (ci + 1) * LQB)
            src = (xh_d, xl_d)[hl]
            dst = (xh_sb, xl_sb)[hl]
            nc.sync.dma_start(out=dst[:, :, sl], in_=src[:, :, sl])

        nc.sync.dma_start(out=wq_sb[:, 0], in_=wq_d[0])
        load_x_chunk(0, 0)
        nc.sync.dma_start(out=wk_sb[:, 0], in_=wk_d[0])
        nc.sync.dma_start(out=bq_sb[:], in_=bq_d)
        nc.sync.dma_start(out=bk_sb[:], in_=bk_d)
        load_x_chunk(0, 1)
        nc.sync.dma_start(out=wq_sb[:, 1], in_=wq_d[1])
        nc.sync.dma_start(out=wk_sb[:, 1], in_=wk_d[1])
        nc.sync.dma_start(out=wv_sb[:, 0], in_=wv_d[0])
        nc.sync.dma_start(out=wv_sb[:, 1], in_=wv_d[1])
        bv_b = bass.AP(tensor=bv_d.tensor, offset=bv_d.offset,
                       ap=[[0, P]] + list(bv_d.ap))
        nc.gpsimd.dma_start(out=bv_bc[:], in_=bv_b)
        nc.sync.dma_start(out=mask_sb[:], in_=mask_d)
        load_x_chunk(1, 0)
        load_x_chunk(1, 1)
        for pair in range(2):
            nc.sync.dma_start(out=wo_sb[:, pair], in_=wo_d[pair])
        for ci in range(2, NLQ):
            load_x_chunk(ci, 0)
            load_x_chunk(ci, 1)

        # ---- persistent work tiles ----------------------------------------
        qt_t = [pers.tile([P, L], BF16, tag=f"qt{p}", name=f"qt{p}") for p in range(2)]
        kt_t = [pers.tile([P, L], BF16, tag=f"kt{p}", name=f"kt{p}") for p in range(2)]
        vx_t = [pers.tile([P, NKB, 2, DH + 1], BF16, tag=f"vx{p}", name=f"vx{p}")
                for p in range(2)]
        osl_t = [pers.tile([P, NKB, P], BF16, tag=f"osl{p}", name=f"osl{p}") for p in range(2)]
        otT_t = [pers.tile([P, NKB, P], BF16, tag=f"otT{p}", name=f"otT{p}") for p in range(2)]

        for _it in range(iters):
            for pair in range(2):
                nc.gpsimd.memset(vx_t[pair][:], WS)

            # (w-term, x-term) for the compensated product
            TERMS = ((0, xh_sb), (1, xh_sb), (0, xl_sb))

            def qk_proj(pair, which, ci):
                w_sb, b_sb, dst = (
                    (wq_sb, bq_sb, qt_t[pair]) if which == 0
                    else (wk_sb, bk_sb, kt_t[pair]))
                sl = slice(ci * LQB, (ci + 1) * LQB)
                acc = ps.tile([P, LQB], F32, tag="acc", bufs=2)
                n = 0
                for wt, x_sb in TERMS:
                    for g2 in range(4):
                        nc.tensor.matmul(
                            acc[:],
                            w_sb[:, wt, g2, :, pair, :],
                            x_sb[:, 2 * g2:2 * g2 + 2, sl],
                            start=(n == 0), stop=(n == 11), perf_mode=DR)
                        n += 1
                nc.vector.tensor_scalar_add(dst[:, sl], acc[:],
                                            b_sb[:, pair:pair + 1])

            def v_proj(j):
                acc = ps.tile([P, 2 * P], F32, tag="acc", bufs=2)
                jsl = slice(j * P, (j + 1) * P)
                n = 0
                for wt, x_sb in TERMS:
                    for g2 in range(4):
                        nc.tensor.matmul(
                            acc[:],
                            x_sb[:, 2 * g2:2 * g2 + 2, jsl],
                            wv_sb[:, wt, g2, :, :],
                            start=(n == 0), stop=(n == 11), perf_mode=DR)
                        n += 1
                for pair in range(2):
                    for h in range(2):
                        c0 = pair * P + h * DH
                        nc.vector.tensor_add(
                            vx_t[pair][:, j, h, 0:DH],
                            acc[:, c0:c0 + DH], bv_bc[:, c0:c0 + DH])

            def s_group(pair, h, ci, jp):
                """S matmuls + exp (+ masks) for pair-tile jp; returns pt."""
                qt, kt = qt_t[pair], kt_t[pair]
                hp = h * DH
                s = ps.tile([P, 2, LQB], F32, tag="s", bufs=2)
                pt = work.tile([P, 2, LQB], BF16, tag="pt", bufs=6)
                for jj in range(2):
                    j = 2 * jp + jj
                    m = j - 4 * ci
                    off = max(0, m) * P
                    nc.tensor.matmul(
                        s[:, jj, off:LQB],
                        kt[hp:hp + DH, j * P:(j + 1) * P],
                        qt[hp:hp + DH, ci * LQB + off:(ci + 1) * LQB],
                        start=True, stop=True)
                if jp < 2 * ci:
                    nc.scalar.activation(pt[:], s[:], AF.Exp, scale=SC)
                else:
                    for jj in range(2):
                        j = 2 * jp + jj
                        m = j - 4 * ci
                        off = max(0, m) * P
                        nc.scalar.activation(pt[:, jj, off:LQB],
                                             s[:, jj, off:LQB],
                                             AF.Exp, scale=SC)
                        if m >= 0:
                            nc.vector.tensor_mul(
                                pt[:, jj, off:LQB], pt[:, jj, off:LQB],
                                mask_sb[:, m, off:LQB])
                return pt

            def pv_group(pair, h, ci, jp, pt, ot):
                # ot is one PSUM bank: hardware start zeroes the whole bank,
                # so the (head, chunk) group has exactly one start (first
                # matmul) and one stop (last matmul).
                vx = vx_t[pair]
                for jj in range(2):
                    j = 2 * jp + jj
                    m = j - 4 * ci
                    for sb in range(max(0, m), 4):
                        nc.tensor.matmul(
                            ot[:, sb, 0:DH + 1],
                            pt[:, jj, sb * P:(sb + 1) * P],
                            vx[:, j, h, :],
                            start=(j == 0 and sb == 0),
                            stop=(j == 4 * ci + 3 and sb == 3),
                            skip_group_check=True)

            def normalize(pair, h, ci, ot):
                hp = h * DH
                rec = work.tile([P, 4], F32, tag="rec", bufs=2)
                nc.vector.reciprocal(rec[:], ot[:, :, DH])
                for sb in range(4):
                    nc.vector.tensor_scalar_mul(
                        osl_t[pair][:, 4 * ci + sb, hp:hp + DH],
                        ot[:, sb, 0:DH], rec[:, sb:sb + 1])

            def outproj_half(lb, half, osb):
                acc = ps.tile([P, LQB], F32, tag="acc", bufs=2)
                for pair in range(2):
                    nc.tensor.matmul(
                        acc[:],
                        otT_t[pair][:, lb, :],
                        wo_sb[:, pair, half * LQB:(half + 1) * LQB],
                        start=(pair == 0), stop=(pair == 1))
                nc.vector.tensor_copy(osb[:, half * LQB:(half + 1) * LQB],
                                      acc[:])

            def outproj_units(ci):
                units = []
                for lb in range(4 * ci, 4 * ci + 4):
                    osb = work.tile([P, D], BF16, tag="osb", bufs=4,
                                    name=f"osb{lb}")
                    for half in range(2):
                        def unit(l=lb, o=osb, hf=half):
                            outproj_half(l, hf, o)
                            nc.sync.dma_start(
                                out=out[l * P:(l + 1) * P,
                                        hf * LQB:(hf + 1) * LQB],
                                in_=o[:, hf * LQB:(hf + 1) * LQB])
                        units.append(unit)
                return units

            # prologue: chunk 0 projections for pair 0 only; pair 1 comes
            # through the filler queue during pair-0 attention. Q and K are
            # staged hi-terms-first so K's hi matmuls overlap the xl DMA.
            pro_accs = []
            for which in range(2):
                w_sb = (wq_sb, wk_sb)[which]
                acc = ps.tile([P, LQB], F32, tag="acc", bufs=2,
                              name=f"proacc{which}")
                for g2 in range(4):
                    nc.tensor.matmul(
                        acc[:], w_sb[:, 0, g2, :, 0, :],
                        xh_sb[:, 2 * g2:2 * g2 + 2, 0:LQB],
                        start=(g2 == 0), stop=False, perf_mode=DR)
                pro_accs.append(acc)
            for which in range(2):
                w_sb, b_sb, dst = ((wq_sb, bq_sb, qt_t[0]),
                                   (wk_sb, bk_sb, kt_t[0]))[which]
                acc = pro_accs[which]
                n = 0
                for wt, x_sb in ((1, xh_sb), (0, xl_sb)):
                    for g2 in range(4):
                        nc.tensor.matmul(
                            acc[:], w_sb[:, wt, g2, :, 0, :],
                            x_sb[:, 2 * g2:2 * g2 + 2, 0:LQB],
                            start=False, stop=(n == 7), perf_mode=DR)
                        n += 1
                nc.vector.tensor_scalar_add(dst[:, 0:LQB], acc[:],
                                            b_sb[:, 0:1])
            for j in range(4):
                v_proj(j)

            # steady state: per chunk, the two head-streams of each pair are
            # interleaved at pair-tile granularity with PV pipelined one
            # round behind S, and a filler queue (next-chunk projections,
            # prev-chunk output projection) feeds the PE stream's exp-wait
            # windows.
            fillers = [lambda: qk_proj(1, 0, 0), lambda: qk_proj(1, 1, 0)]
            deferred = []
            quota = [0.0]

            def drain(slots_left, rate=1.0):
                # spread remaining fillers over remaining drain slots; rate>1
                # front-loads (for units with a chunk-boundary deadline)
                quota[0] += rate * len(fillers) / max(1.0, slots_left)
                while quota[0] >= 1.0 and fillers:
                    quota[0] -= 1.0
                    fillers.pop(0)()

            for ci in range(NLQ):
                nxt = ci + 1
                if nxt < NLQ:
                    for pair in range(2):
                        fillers.append(lambda p=pair: qk_proj(p, 0, nxt))
                        fillers.append(lambda p=pair: qk_proj(p, 1, nxt))
                    for j in range(4 * nxt, 4 * nxt + 4):
                        fillers.append(lambda jj=j: v_proj(jj))
                if ci == NLQ - 1:
                    # late chunks are exp-bound and filler-starved: feed them
                    # the deferred output-projection units
                    fillers.extend(deferred)
                    deferred = []
                nrounds = 2 * ci + 2
                slots = 4 * nrounds
                for pair in range(2):
                    ot_h = [ps.tile([P, 4, P], F32, tag="ot", name=f"ot{h}", bufs=2)
                            for h in range(2)]
                    pt_prev = [None, None]
                    for jp in range(nrounds):
                        for h in range(2):
                            pt = s_group(pair, h, ci, jp)
                            if pt_prev[h] is not None:
                                pv_group(pair, h, ci, jp - 1, pt_prev[h],
                                         ot_h[h])
                            pt_prev[h] = pt
                            drain(slots, 1.0)
                            slots -= 1
                    for h in range(2):
                        pv_group(pair, h, ci, nrounds - 1, pt_prev[h], ot_h[h])
                        normalize(pair, h, ci, ot_h[h])
                    for qb in range(4 * ci, 4 * ci + 2):
                        nc.sync.dma_start(out=otT_t[pair][:, qb, :],
                                          in_=osl_t[pair][:, qb, :],
                                          transpose=True)
                    nc.sync.dma_start(
                        out=otT_t[pair][:, 4 * ci + 2:4 * ci + 4, :],
                        in_=osl_t[pair][:, 4 * ci + 2:4 * ci + 4, :],
                        transpose=True)
                while fillers:
                    fillers.pop(0)()
                if ci >= 2:
                    fillers.extend(outproj_units(ci))
                else:
                    deferred.extend(outproj_units(ci))
            while fillers:
                fillers.pop(0)()

        if dbg:
            for name_, src, dst in (("qt", qt_t[0], dbg_qt),
                                    ("kt", kt_t[0], dbg_kt),
                                    ("vx", vx_t[0], dbg_vx),
                                    ("osl", osl_t[0], dbg_osl),
                                    ("otT", otT_t[0], dbg_otT)):
                tmp = work.tile(list(src.shape), F32, tag=f"dbg{name_}",
                                name=f"dbg{name_}")
                nc.vector.tensor_copy(tmp[:], src[:])
                nc.sync.dma_start(out=dst, in_=tmp[:])

    nc.compile()
    return nc


_CACHE = {}


def _get_nc(mm_dt=None, iters=1):
    key = iters
    if key not in _CACHE:
        _CACHE[key] = build_module(iters)
    return _CACHE[key]


def _split_fp8(a):
    hi = a.astype(E4M3)
    lo = (a - hi.astype(np.float32)).astype(E4M3)
    return hi, lo


def _make_in_maps(x, causal_mask, wq, bq, wk, bk, wv, bv, wo):
    x = np.asarray(x, np.float32)
    cm = np.asarray(causal_mask)
    # mask tile m (for k-block j = 4i+m): keep[p, c] = (c >= 128m + p)
    mt = np.empty((P, 4, LQB), np.float32)
    for m in range(4):
        mt[:, m, :] = (~cm[0, 0, 0:LQB, m * P:(m + 1) * P]).T
    mt = mt.astype(BF16NP)

    wq = np.asarray(wq, np.float32)
    wk = np.asarray(wk, np.float32)
    wv = np.asarray(wv, np.float32)
    wo = np.asarray(wo, np.float32)
    bq = np.asarray(bq, np.float32)
    bk = np.asarray(bk, np.float32)
    bv = np.asarray(bv, np.float32)

    in_maps = []
    for c in range(N_CORES):
        b = c // 4
        g = c % 4
        cols = slice(256 * g, 256 * (g + 1))

        xt = np.ascontiguousarray(
            x[b].T.reshape(KD, P, L).transpose(1, 0, 2))
        xhi, xlo = _split_fp8(xt)

        def pack_qk(w):
            # [D, 256] -> [p, g2, slot, pair, m], scaled
            a = (w[:, cols] * WS).reshape(4, 2, P, 2, P).transpose(2, 0, 1, 3, 4)
            hi, lo = _split_fp8(np.ascontiguousarray(a))
            return np.stack([hi, lo])

        def pack_v(w):
            a = (w[:, cols] * WS).reshape(4, 2, P, 2 * P).transpose(2, 0, 1, 3)
            hi, lo = _split_fp8(np.ascontiguousarray(a))
            return np.stack([hi, lo])

        in_maps.append({
            "xh": xhi,
            "xl": xlo,
            "wq8": pack_qk(wq),
            "wk8": pack_qk(wk),
            "wv8": pack_v(wv),
            "wo": np.ascontiguousarray(
                wo[cols, :].reshape(2, P, D)).astype(BF16NP),
            "bq": np.ascontiguousarray((bq[cols] * WS).reshape(2, P).T),
            "bk": np.ascontiguousarray((bk[cols] * WS).reshape(2, P).T),
            "bv": np.ascontiguousarray(bv[cols] * WS),
            "mask": mt,
        })
    return in_maps


def run(inputs, trace=False, mm_dt=None, iters=1, **kw):
    nc = _get_nc(mm_dt, iters)
    in_maps = _make_in_maps(
        inputs["x"], inputs["causal_mask"], inputs["wq"], inputs["bq"],
        inputs["wk"], inputs["bk"], inputs["wv"], inputs["bv"], inputs["wo"])
    res = run_bass_kernel_spmd(nc, in_maps, list(range(N_CORES)),
                               trace=trace, **kw)
    bo = np.asarray(inputs["bo"], np.float32)
    out = np.zeros((B, L, D), np.float32)
    for c in range(N_CORES):
        out[c // 4] += res.results[c]["out"].astype(np.float32)
    out += bo[None, None, :]
    return out, res


def kernel(**inputs):
    out, _ = run(inputs)
    return out


# revision 31
# speedup vs baseline: 1.6182x; 1.0034x over previous
"""Multi-head attention (B=2, L=2048, D=1024, H=16) on 8 TRN2 NeuronCores.

Sharding: core c handles batch b=c//4 and heads 4*(c%4) .. 4*(c%4)+3
(tensor-parallel over heads x data-parallel over batch). Each core computes a
partial [L, D] output (its heads' contribution through wo); the host sums the
4 partials per batch and adds bo.

Device-side design:
  - QKV projections run in fp8-e4m3 DoubleRow matmuls (0.5 cyc/row, two
    128-row contractions per instruction). Accuracy is preserved with a
    compensated split prepared on the host: x = xh + xl, w = wh + wl (wl/xl
    are the fp8 quantization residuals), and x@w is computed as
    xh@wh + xl@wh + xh@wl (the dropped xl@wl term is ~1e-3 relative).
    Weights are pre-scaled by 32 so their uniform(-1/32,1/32) range stays in
    fp8 normal range; biases are pre-scaled to match, and the V "ones"
    column carries the same scale so softmax normalization cancels it.
  - Q^T/K^T [2*Dh, L]; S^T [k, q] blocks of [128, 512] with exp on paired
    2-bank PSUM tiles; causal handled by skipping k-blocks above the
    diagonal, shrinking diagonal tiles to their unmasked column range, and
    bf16 multiplicative mask tiles for the intra-block triangles.
  - PV runs transposed-back: O [q, dh] via lhsT = P^T slice (full 128-wide
    stationary, N=65 moving V+ones) which halves PV row count vs the
    [dh, q] orientation and makes the softmax denominator per-partition
    (one reciprocal + tensor_scalar per q-subblock, no PE broadcast).
  - O [q, dh] -> O^T via XBAR DMA transposes (SBUF->SBUF, 14ns/tile).
  - Projections/attention/output are software-pipelined per 512-row q-chunk:
    within a chunk the two head-streams of a pair interleave at pair-tile
    granularity with PV one round behind S (hiding exp latency), and a
    filler queue spreads next-chunk projections and deferred output
    projections into the PE stream's exp-wait windows. Output-projection
    units of early chunks are deferred to the exp-bound late chunks, which
    would otherwise starve the PE. The partial output is written bf16 (the
    host accumulates cores in f32).
"""

from contextlib import ExitStack

import numpy as np
import ml_dtypes

import concourse.bass as bass
import concourse.mybir as mybir
import concourse.tile as tile
from concourse import bacc
from concourse.bass_utils import run_bass_kernel_spmd

B, L, D, H = 2, 2048, 1024, 16
DH = D // H          # 64
P = 128              # partitions
NPAIR = 2            # head pairs per core (4 heads)
LQB = 512            # q chunk
NLQ = L // LQB       # 4
NKB = L // P         # 16 k blocks
KD = D // P          # 8 contraction blocks over D
N_CORES = 8
WS = 32.0            # host-side weight scale (fp8 range)
SC = (1.0 / np.sqrt(DH)) / (WS * WS)   # exp scale: undo WS^2 in scores

F32 = mybir.dt.float32
BF16 = mybir.dt.bfloat16
FP8 = mybir.dt.float8e4
AF = mybir.ActivationFunctionType
DR = mybir.MatmulPerfMode.DoubleRow
E4M3 = ml_dtypes.float8_e4m3
BF16NP = ml_dtypes.bfloat16


def build_module(iters=1, dbg=False):
    nc = bacc.Bacc("TRN2", target_bir_lowering=False, debug=False,
                   num_devices=N_CORES)
    if dbg:
        dbg_qt = nc.dram_tensor("dbg_qt", [P, L], F32, kind="ExternalOutput").ap()
        dbg_kt = nc.dram_tensor("dbg_kt", [P, L], F32, kind="ExternalOutput").ap()
        dbg_vx = nc.dram_tensor("dbg_vx", [P, NKB, 2, DH + 1], F32,
                                kind="ExternalOutput").ap()
        dbg_osl = nc.dram_tensor("dbg_osl", [P, NKB, P], F32,
                                 kind="ExternalOutput").ap()
        dbg_otT = nc.dram_tensor("dbg_otT", [P, NKB, P], F32,
                                 kind="ExternalOutput").ap()

    xh_d = nc.dram_tensor("xh", [P, KD, L], FP8, kind="ExternalInput").ap()
    xl_d = nc.dram_tensor("xl", [P, KD, L], FP8, kind="ExternalInput").ap()
    # [hl, p, g2, slot, pair, m]
    wq_d = nc.dram_tensor("wq8", [2, P, 4, 2, 2, P], FP8, kind="ExternalInput").ap()
    wk_d = nc.dram_tensor("wk8", [2, P, 4, 2, 2, P], FP8, kind="ExternalInput").ap()
    # [hl, p, g2, slot, ch]
    wv_d = nc.dram_tensor("wv8", [2, P, 4, 2, 2 * P], FP8, kind="ExternalInput").ap()
    wo_d = nc.dram_tensor("wo", [2, P, D], BF16, kind="ExternalInput").ap()
    bq_d = nc.dram_tensor("bq", [P, 2], F32, kind="ExternalInput").ap()
    bk_d = nc.dram_tensor("bk", [P, 2], F32, kind="ExternalInput").ap()
    bv_d = nc.dram_tensor("bv", [2 * P], F32, kind="ExternalInput").ap()
    mask_d = nc.dram_tensor("mask", [P, 4, LQB], BF16, kind="ExternalInput").ap()
    out = nc.dram_tensor("out", [L, D], BF16, kind="ExternalOutput").ap()

    with tile.TileContext(nc) as tc, ExitStack() as ctx:
        ctx.enter_context(
            nc.allow_low_precision(reason="fp8/bf16 matmul data path"))
        consts = ctx.enter_context(tc.tile_pool(name="consts", bufs=1))
        pers = ctx.enter_context(tc.tile_pool(name="pers", bufs=1))
        work = ctx.enter_context(tc.tile_pool(name="work", bufs=1))
        ps = ctx.enter_context(tc.tile_pool(name="ps", bufs=1, space="PSUM"))

        # ---- const tiles ---------------------------------------------------
        xh_sb = consts.tile([P, KD, L], FP8, tag="xh")
        xl_sb = consts.tile([P, KD, L], FP8, tag="xl")
        wq_sb = consts.tile([P, 2, 4, 2, 2, P], FP8, tag="wq")
        wk_sb = consts.tile([P, 2, 4, 2, 2, P], FP8, tag="wk")
        wv_sb = consts.tile([P, 2, 4, 2, 2 * P], FP8, tag="wv")
        wo_sb = consts.tile([P, 2, D], BF16, tag="wo")
        bq_sb = consts.tile([P, 2], F32, tag="bq")
        bk_sb = consts.tile([P, 2], F32, tag="bk")
        bv_bc = consts.tile([P, 2 * P], F32, tag="bv")
        mask_sb = consts.tile([P, 4, LQB], BF16, tag="mask")

        # DMA order matters: weights for chunk-0 QK first, then x chunk
        # slabs interleaved with the tensors each chunk unlocks.
        def load_x_chunk(ci, hl):
            sl = slice(ci * LQB, (ci + 1) * LQB)
            src = (xh_d, xl_d)[hl]
            dst = (xh_sb, xl_sb)[hl]
            nc.sync.dma_start(out=dst[:, :, sl], in_=src[:, :, sl])

        nc.sync.dma_start(out=wq_sb[:, 0], in_=wq_d[0])
        load_x_chunk(0, 0)
        nc.sync.dma_start(out=wk_sb[:, 0], in_=wk_d[0])
        nc.sync.dma_start(out=bq_sb[:], in_=bq_d)
        nc.sync.dma_start(out=bk_sb[:], in_=bk_d)
        load_x_chunk(0, 1)
        nc.sync.dma_start(out=wq_sb[:, 1], in_=wq_d[1])
        nc.sync.dma_start(out=wk_sb[:, 1], in_=wk_d[1])
        nc.sync.dma_start(out=wv_sb[:, 0], in_=wv_d[0])
        nc.sync.dma_start(out=wv_sb[:, 1], in_=wv_d[1])
        bv_b = bass.AP(tensor=bv_d.tensor, offset=bv_d.offset,
                       ap=[[0, P]] + list(bv_d.ap))
        nc.gpsimd.dma_start(out=bv_bc[:], in_=bv_b)
        nc.sync.dma_start(out=mask_sb[:], in_=mask_d)
        load_x_chunk(1, 0)
        load_x_chunk(1, 1)
        for pair in range(2):
            nc.sync.dma_start(out=wo_sb[:, pair], in_=wo_d[pair])
        for ci in range(2, NLQ):
            load_x_chunk(ci, 0)
            load_x_chunk(ci, 1)

        # ---- persistent work tiles ----------------------------------------
        qt_t = [pers.tile([P, L], BF16, tag=f"qt{p}", name=f"qt{p}") for p in range(2)]
        kt_t = [pers.tile([P, L], BF16, tag=f"kt{p}", name=f"kt{p}") for p in range(2)]
        vx_t = [pers.tile([P, NKB, 2, DH + 1], BF16, tag=f"vx{p}", name=f"vx{p}")
                for p in range(2)]
        osl_t = [pers.tile([P, NKB, P], BF16, tag=f"osl{p}", name=f"osl{p}") for p in range(2)]
        otT_t = [pers.tile([P, NKB, P], BF16, tag=f"otT{p}", name=f"otT{p}") for p in range(2)]

        for _it in range(iters):
            for pair in range(2):
                nc.gpsimd.memset(vx_t[pair][:], WS)

            # (w-term, x-term) for the compensated product
            TERMS = ((0, xh_sb), (1, xh_sb), (0, xl_sb))

            def qk_proj(pair, which, ci):
                w_sb, b_sb, dst = (
                    (wq_sb, bq_sb, qt_t[pair]) if which == 0
                    else (wk_sb, bk_sb, kt_t[pair]))
                sl = slice(ci * LQB, (ci + 1) * LQB)
                acc = ps.tile([P, LQB], F32, tag="acc", bufs=2)
                n = 0
                for wt, x_sb in TERMS:
                    for g2 in range(4):
                        nc.tensor.matmul(
                            acc[:],
                            w_sb[:, wt, g2, :, pair, :],
                            x_sb[:, 2 * g2:2 * g2 + 2, sl],
                            start=(n == 0), stop=(n == 11), perf_mode=DR)
                        n += 1
                nc.vector.tensor_scalar_add(dst[:, sl], acc[:],
                                            b_sb[:, pair:pair + 1])

            def v_proj(j):
                acc = ps.tile([P, 2 * P], F32, tag="acc", bufs=2)
                jsl = slice(j * P, (j + 1) * P)
                n = 0
                for wt, x_sb in TERMS:
                    for g2 in range(4):
                        nc.tensor.matmul(
                            acc[:],
                            x_sb[:, 2 * g2:2 * g2 + 2, jsl],
                            wv_sb[:, wt, g2, :, :],
                            start=(n == 0), stop=(n == 11), perf_mode=DR)
                        n += 1
                for pair in range(2):
                    for h in range(2):
                        c0 = pair * P + h * DH
                        nc.vector.tensor_add(
                            vx_t[pair][:, j, h, 0:DH],
                            acc[:, c0:c0 + DH], bv_bc[:, c0:c0 + DH])

            def s_group(pair, h, ci, jp):
                """S matmuls + exp (+ masks) for pair-tile jp; returns pt."""
                qt, kt = qt_t[pair], kt_t[pair]
                hp = h * DH
                s = ps.tile([P, 2, LQB], F32, tag="s", bufs=2)
                pt = work.tile([P, 2, LQB], BF16, tag="pt", bufs=8)
                poff = 0 if jp <= 2 * ci else 2 * P
                for jj in range(2):
                    j = 2 * jp + jj
                    off = poff if jp >= 2 * ci else 0
                    nc.tensor.matmul(
                        s[:, jj, off:LQB],
                        kt[hp:hp + DH, j * P:(j + 1) * P],
                        qt[hp:hp + DH, ci * LQB + off:(ci + 1) * LQB],
                        start=True, stop=True)
                if jp < 2 * ci:
                    nc.scalar.activation(pt[:], s[:], AF.Exp, scale=SC)
                else:
                    nc.scalar.activation(pt[:, :, poff:LQB],
                                         s[:, :, poff:LQB], AF.Exp, scale=SC)
                    for jj in range(2):
                        m = 2 * jp + jj - 4 * ci
                        if m >= 0:
                            nc.vector.tensor_mul(
                                pt[:, jj, poff:LQB], pt[:, jj, poff:LQB],
                                mask_sb[:, m, poff:LQB])
                return pt

            def pv_group(pair, h, ci, jp, pt, ot):
                # ot is one PSUM bank: hardware start zeroes the whole bank,
                # so the (head, chunk) group has exactly one start (first
                # matmul) and one stop (last matmul).
                vx = vx_t[pair]
                for jj in range(2):
                    j = 2 * jp + jj
                    m = j - 4 * ci
                    for sb in range(max(0, m), 4):
                        nc.tensor.matmul(
                            ot[:, sb, 0:DH + 1],
                            pt[:, jj, sb * P:(sb + 1) * P],
                            vx[:, j, h, :],
                            start=(j == 0 and sb == 0),
                            stop=(j == 4 * ci + 3 and sb == 3),
                            skip_group_check=True)

            def normalize(pair, h, ci, ot):
                hp = h * DH
                rec = work.tile([P, 4], F32, tag="rec", bufs=4)
                nc.vector.reciprocal(rec[:], ot[:, :, DH])
                for sb in range(4):
                    nc.vector.tensor_scalar_mul(
                        osl_t[pair][:, 4 * ci + sb, hp:hp + DH],
                        ot[:, sb, 0:DH], rec[:, sb:sb + 1])

            def outproj_half(lb, half, osb):
                acc = ps.tile([P, LQB], F32, tag="acc", bufs=2)
                for pair in range(2):
                    nc.tensor.matmul(
                        acc[:],
                        otT_t[pair][:, lb, :],
                        wo_sb[:, pair, half * LQB:(half + 1) * LQB],
                        start=(pair == 0), stop=(pair == 1))
                nc.vector.tensor_copy(osb[:, half * LQB:(half + 1) * LQB],
                                      acc[:])

            def outproj_units(ci):
                units = []
                for lb in range(4 * ci, 4 * ci + 4):
                    osb = work.tile([P, D], BF16, tag="osb", bufs=4,
                                    name=f"osb{lb}")
                    for half in range(2):
                        def unit(l=lb, o=osb, hf=half):
                            outproj_half(l, hf, o)
                            nc.sync.dma_start(
                                out=out[l * P:(l + 1) * P,
                                        hf * LQB:(hf + 1) * LQB],
                                in_=o[:, hf * LQB:(hf + 1) * LQB])
                        units.append(unit)
                return units

            # prologue: chunk 0 projections for pair 0 only; pair 1 comes
            # through the filler queue during pair-0 attention. Q and K are
            # staged hi-terms-first so K's hi matmuls overlap the xl DMA.
            pro_accs = []
            for which in range(2):
                w_sb = (wq_sb, wk_sb)[which]
                acc = ps.tile([P, LQB], F32, tag="acc", bufs=2,
                              name=f"proacc{which}")
                for g2 in range(4):
                    nc.tensor.matmul(
                        acc[:], w_sb[:, 0, g2, :, 0, :],
                        xh_sb[:, 2 * g2:2 * g2 + 2, 0:LQB],
                        start=(g2 == 0), stop=False, perf_mode=DR)
                pro_accs.append(acc)
            for which in range(2):
                w_sb, b_sb, dst = ((wq_sb, bq_sb, qt_t[0]),
                                   (wk_sb, bk_sb, kt_t[0]))[which]
                acc = pro_accs[which]
                n = 0
                for wt, x_sb in ((1, xh_sb), (0, xl_sb)):
                    for g2 in range(4):
                        nc.tensor.matmul(
                            acc[:], w_sb[:, wt, g2, :, 0, :],
                            x_sb[:, 2 * g2:2 * g2 + 2, 0:LQB],
                            start=False, stop=(n == 7), perf_mode=DR)
                        n += 1
                nc.vector.tensor_scalar_add(dst[:, 0:LQB], acc[:],
                                            b_sb[:, 0:1])
            for j in range(4):
                v_proj(j)

            # steady state: per chunk, the two head-streams of each pair are
            # interleaved at pair-tile granularity with PV pipelined one
            # round behind S, and a filler queue (next-chunk projections,
            # prev-chunk output projection) feeds the PE stream's exp-wait
            # windows.
            fillers = [lambda: qk_proj(1, 0, 0), lambda: qk_proj(1, 1, 0)]
            deferred = []
            quota = [0.0]

            def drain(slots_left, rate=1.0):
                # spread remaining fillers over remaining drain slots; rate>1
                # front-loads (for units with a chunk-boundary deadline)
                quota[0] += rate * len(fillers) / max(1.0, slots_left)
                while quota[0] >= 1.0 and fillers:
                    quota[0] -= 1.0
                    fillers.pop(0)()

            for ci in range(NLQ):
                nxt = ci + 1
                if nxt < NLQ:
                    for pair in range(2):
                        fillers.append(lambda p=pair: qk_proj(p, 0, nxt))
                        fillers.append(lambda p=pair: qk_proj(p, 1, nxt))
                    for j in range(4 * nxt, 4 * nxt + 4):
                        fillers.append(lambda jj=j: v_proj(jj))
                if ci == NLQ - 1:
                    # late chunks are exp-bound and filler-starved: feed them
                    # the deferred output-projection units
                    fillers.extend(deferred)
                    deferred = []
                nrounds = 2 * ci + 2
                slots = 4 * nrounds
                for pair in range(2):
                    ot_h = [ps.tile([P, 4, P], F32, tag="ot", name=f"ot{h}", bufs=2)
                            for h in range(2)]
                    pt_prev = [None, None]
                    for jp in range(nrounds):
                        for h in range(2):
                            pt = s_group(pair, h, ci, jp)
                            if pt_prev[h] is not None:
                                pv_group(pair, h, ci, jp - 1, pt_prev[h],
                                         ot_h[h])
                            pt_prev[h] = pt
                            drain(slots, 1.0)
                            slots -= 1
                    for h in range(2):
                        pv_group(pair, h, ci, nrounds - 1, pt_prev[h], ot_h[h])
                        normalize(pair, h, ci, ot_h[h])
                    for qb in range(4 * ci, 4 * ci + 2):
                        nc.sync.dma_start(out=otT_t[pair][:, qb, :],
                                          in_=osl_t[pair][:, qb, :],
                                          transpose=True)
                    nc.sync.dma_start(
                        out=otT_t[pair][:, 4 * ci + 2:4 * ci + 4, :],
                        in_=osl_t[pair][:, 4 * ci + 2:4 * ci + 4, :],
                        transpose=True)
                while fillers:
                    fillers.pop(0)()
                if ci >= 2:
                    fillers.extend(outproj_units(ci))
                else:
                    deferred.extend(outproj_units(ci))
            while fillers:
                fillers.pop(0)()

        if dbg:
            for name_, src, dst in (("qt", qt_t[0], dbg_qt),
                                    ("kt", kt_t[0], dbg_kt),
                                    ("vx", vx_t[0], dbg_vx),
                                    ("osl", osl_t[0], dbg_osl),
                                    ("otT", otT_t[0], dbg_otT)):
                tmp = work.tile(list(src.shape), F32, tag=f"dbg{name_}",
                                name=f"dbg{name_}")
                nc.vector.tensor_copy(tmp[:], src[:])
                nc.sync.dma_start(out=dst, in_=tmp[:])

    nc.compile()
    return nc


_CACHE = {}


def _get_nc(mm_dt=None, iters=1):
    key = iters
    if key not in _CACHE:
        _CACHE[key] = build_module(iters)
    return _CACHE[key]


def _split_fp8(a):
    hi = a.astype(E4M3)
    lo = (a - hi.astype(np.float32)).astype(E4M3)
    return hi, lo


def _make_in_maps(x, causal_mask, wq, bq, wk, bk, wv, bv, wo):
    x = np.asarray(x, np.float32)
    cm = np.asarray(causal_mask)
    # mask tile m (for k-block j = 4i+m): keep[p, c] = (c >= 128m + p)
    mt = np.empty((P, 4, LQB), np.float32)
    for m in range(4):
        mt[:, m, :] = (~cm[0, 0, 0:LQB, m * P:(m + 1) * P]).T
    mt = mt.astype(BF16NP)

    wq = np.asarray(wq, np.float32)
    wk = np.asarray(wk, np.float32)
    wv = np.asarray(wv, np.float32)
    wo = np.asarray(wo, np.float32)
    bq = np.asarray(bq, np.float32)
    bk = np.asarray(bk, np.float32)
    bv = np.asarray(bv, np.float32)

    in_maps = []
    for c in range(N_CORES):
        b = c // 4
        g = c % 4
        cols = slice(256 * g, 256 * (g + 1))

        xt = np.ascontiguousarray(
            x[b].T.reshape(KD, P, L).transpose(1, 0, 2))
        xhi, xlo = _split_fp8(xt)

        def pack_qk(w):
            # [D, 256] -> [p, g2, slot, pair, m], scaled
            a = (w[:, cols] * WS).reshape(4, 2, P, 2, P).transpose(2, 0, 1, 3, 4)
            hi, lo = _split_fp8(np.ascontiguousarray(a))
            return np.stack([hi, lo])

        def pack_v(w):
            a = (w[:, cols] * WS).reshape(4, 2, P, 2 * P).transpose(2, 0, 1, 3)
            hi, lo = _split_fp8(np.ascontiguousarray(a))
            return np.stack([hi, lo])

        in_maps.append({
            "xh": xhi,
            "xl": xlo,
            "wq8": pack_qk(wq),
            "wk8": pack_qk(wk),
            "wv8": pack_v(wv),
            "wo": np.ascontiguousarray(
                wo[cols, :].reshape(2, P, D)).astype(BF16NP),
            "bq": np.ascontiguousarray((bq[cols] * WS).reshape(2, P).T),
            "bk": np.ascontiguousarray((bk[cols] * WS).reshape(2, P).T),
            "bv": np.ascontiguousarray(bv[cols] * WS),
            "mask": mt,
        })
    return in_maps


def run(inputs, trace=False, mm_dt=None, iters=1, **kw):
    nc = _get_nc(mm_dt, iters)
    in_maps = _make_in_maps(
        inputs["x"], inputs["causal_mask"], inputs["wq"], inputs["bq"],
        inputs["wk"], inputs["bk"], inputs["wv"], inputs["bv"], inputs["wo"])
    res = run_bass_kernel_spmd(nc, in_maps, list(range(N_CORES)),
                               trace=trace, **kw)
    bo = np.asarray(inputs["bo"], np.float32)
    out = np.zeros((B, L, D), np.float32)
    for c in range(N_CORES):
        out[c // 4] += res.results[c]["out"].astype(np.float32)
    out += bo[None, None, :]
    return out, res


def kernel(**inputs):
    out, _ = run(inputs)
    return out
